# revision 1
# baseline (speedup 1.0000x reference)
"""GCN NodeAttributeAggregator on 8 Trainium2 NeuronCores.

Strategy (node-sharded, dst-partitioned edges):
  - Host precomputes index metadata: degrees (with self-loops), dinv=rsqrt(deg),
    per-core edge lists partitioned by dst owner, grouped by (dst-block of 128,
    src<32768 vs src>=32768 for int16 gather indices), padded to 128-edge tiles.
  - Device per core: dma_gather rows of a pre-scaled node table (xs = x*dinv),
    scatter-by-matmul: one-hot P matrices (built on DVE via iota + is_equal)
    contract each 128-edge tile into a 128-dst PSUM accumulator.
  - Dense 256x256 matmuls run in float32r (full PE rate) feature-major, with
    PE transposes at layout boundaries.
  - Algebra: GCN layer out = D^-1/2 (A+I) D^-1/2 h W.  Row scaling commutes
    with right matmuls, relu commutes with positive row scaling, and
    agg(h W) = agg(h) W, so:
      L1 (fused W_pre@W1): u' = (scatter(xs) + xs_dst) * dinv;
          g1 = relu(u' @ (W_pre W1) + b1 + rank1(b_pre)) * dinv
      L2: v' = (scatter(g1) + g1_dst) * dinv;
          y  = relu(v' @ W2 + b2) @ W_post + b_post
  - Two SPMD launches; host concatenates g1 slices between them.
"""

import dataclasses
import ml_dtypes
import numpy as np

import concourse.bacc as bacc
import concourse.bass as bass
import concourse.tile as tile
import concourse.mybir as mybir
from concourse.bass_utils import run_bass_kernel_spmd
from concourse.masks import make_identity

P = 128
SCAT_BF16 = True  # bf16 gather tables + P matrices (PE 1 cyc/row + FWL)
NSWQ = 4  # SWDGE queues
GCH = 8  # gather tiles per dma_gather call
f32 = mybir.dt.float32
f32r = mybir.dt.float32r
bf16 = mybir.dt.bfloat16
gdt = bf16 if SCAT_BF16 else f32r
i16 = mybir.dt.int16
i32 = mybir.dt.int32


@dataclasses.dataclass
class Cfg:
    n_nodes: int = 50000
    d: int = 256
    nc: int = 8
    split: int = 32768
    dense_n: int = 512

    @property
    def nloc(self):
        return self.n_nodes // self.nc

    @property
    def nblk(self):
        return (self.nloc + P - 1) // P

    @property
    def npad(self):
        return self.nblk * P

    @property
    def nhi(self):
        # table B spans the last min(32768, n) rows (full int16 window)
        return min(self.split, self.n_nodes)


# ---------------------------------------------------------------- host prep


def _wrap16(vals):
    """[n] -> [128, n//16] int16: value i at [i%16, i//16], replicated x8."""
    w = vals.reshape(-1, 16).T
    return np.tile(w, (8, 1)).astype(np.int16)


def _prep_edges(cfg, src, dst):
    """Partition edges by dst owner; split per dst-block into two gather
    groups (table A = rows [0, split); table B = rows [baseB, n)).  The cut
    is chosen per block at an exact multiple of 128 edges in src-order when
    the overlap window [baseB, split) allows, so group A has no padding.

    Returns (TA, TB, per-core dict of idx/slot planes) with identical
    compile-time schedule (TA, TB) across cores.
    """
    nl, nb = cfg.nloc, cfg.nblk
    baseB = cfg.n_nodes - cfg.nhi
    owner = dst // nl
    loc = dst - owner * nl
    blk = loc // P
    slot = loc - blk * P

    key = owner * nb + blk
    nkeys = cfg.nc * nb
    n_cb = np.bincount(key, minlength=nkeys).reshape(cfg.nc, nb)
    cntlo = np.bincount(key[src < baseB], minlength=nkeys).reshape(cfg.nc, nb)
    cntA = np.bincount(key[src < cfg.split], minlength=nkeys).reshape(cfg.nc, nb)

    lo = -(-cntlo // P).max(axis=0)          # [nb] min feasible TA
    hi = (cntA // P).min(axis=0)             # [nb] max feasible TA
    TA = np.maximum(hi, lo)                  # prefer max A (A has no pad)
    feasible = lo <= hi
    # fallback for infeasible blocks: threshold split at `split`
    TA = np.where(feasible, TA, -(-cntA // P).max(axis=0))
    TB = np.maximum(-(-(n_cb - np.minimum(TA * P, cntA if True else 0)
                        ) // P), 0)  # placeholder, fixed below
    # B count per (c,b): n - A_count; A_count = TA*P if feasible else cntA
    A_cnt = np.where(feasible[None, :], np.minimum(TA[None, :] * P, n_cb),
                     cntA)
    B_cnt = n_cb - A_cnt
    TB = (-(-B_cnt // P)).max(axis=0)

    # global A/B layout: all blocks' A segments first, then all B segments,
    # so gather calls can span block boundaries within one table
    baseA = np.concatenate([[0], np.cumsum(TA * P)])[:-1]
    baseBs = int(TA.sum()) * P + np.concatenate([[0], np.cumsum(TB * P)])[:-1]
    gbase = np.stack([baseA, baseBs], axis=1).astype(np.int64)
    rows_total = int((TA.sum() + TB.sum()) * P)

    # rank edges within (core, block) by src (stable) to apply the cut
    order = np.lexsort((src, key))
    skey = key[order]
    group_start = np.concatenate(
        [[0], np.cumsum(np.bincount(skey, minlength=nkeys))])
    rank = np.arange(len(src)) - group_start[skey]

    ocore = owner[order]
    oblk = blk[order]
    acut = A_cnt[ocore, oblk]
    in_a = rank < acut
    grp = (~in_a).astype(np.int64)
    rowpos = np.where(in_a, rank, rank - acut)
    rows = gbase[oblk, grp] + rowpos
    idxv = (src[order] - grp * baseB).astype(np.int16)
    slotv = slot[order].astype(np.float32)
    assert (src[order][in_a] < cfg.split).all()
    assert (src[order][~in_a] >= baseB).all()

    idx_flat = np.zeros((cfg.nc, rows_total), np.int16)
    slot_flat = np.full((cfg.nc, rows_total), 300.0, np.float32)
    idx_flat[ocore, rows] = idxv
    slot_flat[ocore, rows] = slotv

    per_core = []
    for c in range(cfg.nc):
        idxp = _wrap16(idx_flat[c])
        slotp = slot_flat[c].reshape(-1, P).T.copy()
        per_core.append({"idxp": idxp, "slotp": slotp})
    return TA, TB, per_core


def _wrap_cols(vec, nblk, npad):
    """[npad] -> [128, nblk] with [p, b] = vec[b*128+p]."""
    v = np.zeros(npad, np.float32)
    v[: len(vec)] = vec
    return v.reshape(nblk, P).T.copy()


# ------------------------------------------------------------- device build


def build_launch(cfg, mode, TA, TB, has_bpre=False):
    """mode 1: out = relu(u' @ WA + b1 [+ rank1]) * dinv   (writes g1)
    mode 2: out = relu(v' @ W2 + b2) @ W_post + b_post     (writes y)
    """
    nb, npad, d = cfg.nblk, cfg.npad, cfg.d
    ntiles = int((TA + TB).sum())
    nidxcol = ntiles * 8
    tmaxP = max(int((TA + TB).max()), 1)

    # global A/B tile layout (matches host): A tiles of all blocks first,
    # then all B tiles.  Gather calls span block boundaries, so nearly all
    # calls are full GCH tiles.
    totA = int(TA.sum())
    ta_base = np.concatenate([[0], np.cumsum(TA)])[:-1]
    tb_base = totA + np.concatenate([[0], np.cumsum(TB)])[:-1]

    # gather-call plan per group: (grp, t0, cn, icol); icol = t0 * 8
    calls = []
    for grp, gt0, gcnt in ((0, 0, totA), (1, totA, int(TB.sum()))):
        for c0 in range(0, gcnt, GCH):
            cn = min(GCH, gcnt - c0)
            calls.append((grp, gt0 + c0, cn, (gt0 + c0) * 8))
    assert sum(c[2] for c in calls) == ntiles

    # split the idx plane into chunks at call boundaries so early gathers
    # only wait on the first chunks; A and B ranges are chunked separately
    # and their DMAs interleaved (block 0 needs the head of both ranges)
    acol = totA * 8
    astarts = [c[3] for c in calls if c[0] == 0]
    bstarts = [c[3] for c in calls if c[0] == 1]

    def _cuts(starts, lo, hi, n):
        cs = [lo]
        for k in range(1, n):
            tgt = lo + (hi - lo) * k // n
            best = min(starts, key=lambda x: abs(x - tgt))
            if best > cs[-1]:
                cs.append(best)
        cs.append(hi)
        return list(zip(cs[:-1], cs[1:]))

    a_ch = _cuts(astarts, 0, acol, 3)
    b_ch = _cuts(bstarts, acol, nidxcol, 2) if acol < nidxcol else []
    chunks = []
    for i in range(max(len(a_ch), len(b_ch))):
        if i < len(a_ch):
            chunks.append(a_ch[i])
        if i < len(b_ch):
            chunks.append(b_ch[i])

    nc = bacc.Bacc("TRN2", target_bir_lowering=False, debug=False,
                   num_devices=cfg.nc, num_swdge_queues=NSWQ)

    tablo = nc.dram_tensor("tablo", [cfg.split, d], gdt, kind="ExternalInput")
    loctab = nc.dram_tensor("loctab", [npad, d], gdt, kind="ExternalInput")
    tabhi = nc.dram_tensor("tabhi", [cfg.nhi, d], gdt, kind="ExternalInput")
    idxp_d = nc.dram_tensor("idxp", [P, nidxcol], i16, kind="ExternalInput")
    slotp_d = nc.dram_tensor("slotp", [P, ntiles], f32, kind="ExternalInput")
    dinvw_d = nc.dram_tensor("dinvw", [P, nb], f32, kind="ExternalInput")
    nw = 1 if mode == 1 else 2
    w_d = [nc.dram_tensor(f"w{i}", [d, d], f32r, kind="ExternalInput")
           for i in range(nw)]
    bias_d = [nc.dram_tensor(f"bias{i}", [P, d // P], f32, kind="ExternalInput")
              for i in range(nw)]
    if has_bpre:
        c1rep_d = nc.dram_tensor("c1rep", [P, npad], f32, kind="ExternalInput")
        v1w_d = nc.dram_tensor("v1w", [P, d // P], f32, kind="ExternalInput")
    out_d = nc.dram_tensor("out", [npad, d], bf16, kind="ExternalOutput")

    kd = d // P  # feature k-tiles (2)

    with tile.TileContext(nc) as tc:
        with (
            tc.tile_pool(name="const", bufs=1) as cpool,
            tc.tile_pool(name="gA", bufs=8) as gApool,
            tc.tile_pool(name="gB", bufs=6) as gBpool,
            tc.tile_pool(name="loc", bufs=6) as locpool,
            tc.tile_pool(name="pmat", bufs=3) as ppool,
            tc.tile_pool(name="work", bufs=3) as wpool,
            tc.tile_pool(name="stage", bufs=3) as stpool,
            tc.tile_pool(name="zslab", bufs=2) as zpool,
            tc.tile_pool(name="apsum", bufs=3, space="PSUM") as apsum,
            tc.tile_pool(name="trpsum", bufs=2, space="PSUM") as trpsum,
            tc.tile_pool(name="dpsum", bufs=3, space="PSUM") as dpsum,
        ):
            # ---- warmup: tiny gathers on all queues absorb SWDGE ucode
            # cold-start while the idx planes stream in
            izero = cpool.tile([P, 8], i16)
            nc.gpsimd.memset(izero[:], 0)
            warm = cpool.tile([P, NSWQ, d], gdt)
            for q in range(NSWQ):
                nc.gpsimd.dma_gather(
                    out_ap=warm[:, q:q + 1, :], in_ap=tablo[:],
                    idxs_ap=izero[:],
                    num_idxs=P, num_idxs_reg=P, elem_size=d, queue_num=q)

            # ---- constants (idx planes first: gathers depend on them)
            idxp_t = cpool.tile([P, nidxcol], i16)
            for lo, hi in chunks:
                nc.sync.dma_start(idxp_t[:, lo:hi], idxp_d[:, lo:hi])
            slotp_t = cpool.tile([P, ntiles], f32)
            nc.sync.dma_start(slotp_t[:], slotp_d[:])
            dinvw_t = cpool.tile([P, nb], f32)
            nc.sync.dma_start(dinvw_t[:], dinvw_d[:])
            iota_i = cpool.tile([P, P], i32)
            nc.gpsimd.iota(iota_i[:], pattern=[[1, P]], base=0,
                           channel_multiplier=0)
            iota_f = cpool.tile([P, P], f32)
            nc.vector.tensor_copy(iota_f[:], iota_i[:])
            ident = cpool.tile([P, P], f32)
            make_identity(nc, ident[:])
            ident_g = cpool.tile([P, P], gdt)
            nc.vector.tensor_copy(ident_g[:], ident[:])
            if has_bpre:
                c1rep_t = cpool.tile([P, npad], f32)
                nc.sync.dma_start(c1rep_t[:], c1rep_d[:])
                v1w_t = cpool.tile([P, kd], f32)
                nc.sync.dma_start(v1w_t[:], v1w_d[:])

            # weights/biases are loaded lazily (after block 0's gathers are
            # queued) — first dense slice runs ~6 blocks in
            w_t = [[[None] * kd for _ in range(kd)] for _ in range(nw)]
            bias_t = [None] * nw

            def load_weights():
                for i in range(nw):
                    for k in range(kd):
                        for m in range(kd):
                            wt = cpool.tile([P, P], f32r,
                                            name=f"wt{i}_{k}_{m}",
                                            tag=f"wt{i}_{k}_{m}")
                            nc.sync.dma_start(
                                wt[:],
                                w_d[i][k * P:(k + 1) * P, m * P:(m + 1) * P])
                            w_t[i][k][m] = wt
                    bt = cpool.tile([P, kd], f32, name=f"bt{i}", tag=f"bt{i}")
                    nc.sync.dma_start(bt[:], bias_d[i][:])
                    bias_t[i] = bt

            # feature-major activations, one tile per dense node-slice
            nsl = (npad + cfg.dense_n - 1) // cfg.dense_n
            uT_s = [cpool.tile([P, kd, min(cfg.dense_n, npad - i * cfg.dense_n)],
                               f32r, name=f"uTs{i}", tag=f"uTs{i}")
                    for i in range(nsl)]

            def emit_dense(s):
                s0 = s * cfg.dense_n
                ns = min(cfg.dense_n, npad - s0)
                pz = [dpsum.tile([P, ns], f32, space="PSUM", tag="dps",
                                 name=f"pz{s}_{dt}") for dt in range(kd)]
                for dt in range(kd):
                    for m in range(kd):
                        nc.tensor.matmul(
                            pz[dt][:], lhsT=w_t[0][m][dt][:],
                            rhs=uT_s[s][:, m, 0:ns],
                            start=(m == 0), stop=(m == kd - 1))
                if has_bpre:
                    for dt in range(kd):
                        tmp = wpool.tile([P, cfg.dense_n], f32, tag="r1")
                        nc.vector.tensor_scalar_mul(
                            tmp[:, 0:ns], c1rep_t[:, s0:s0 + ns],
                            v1w_t[:, dt:dt + 1])
                        nc.vector.tensor_tensor(
                            out=pz[dt][:], in0=pz[dt][:], in1=tmp[:, 0:ns],
                            op=mybir.AluOpType.add)

                if mode == 1:
                    zr = zpool.tile([P, kd, cfg.dense_n], f32, tag="zr")
                    for dt in range(kd):
                        nc.scalar.activation(
                            zr[:, dt, 0:ns], pz[dt][:],
                            mybir.ActivationFunctionType.Relu,
                            bias=bias_t[0][:, dt:dt + 1], scale=1.0)
                    final = zr
                else:
                    rT = zpool.tile([P, kd, cfg.dense_n], f32r, tag="zr")
                    for dt in range(kd):
                        nc.scalar.activation(
                            rT[:, dt, 0:ns], pz[dt][:],
                            mybir.ActivationFunctionType.Relu,
                            bias=bias_t[0][:, dt:dt + 1], scale=1.0)
                    py = [dpsum.tile([P, ns], f32, space="PSUM", tag="dps",
                                     name=f"py{s}_{dt}") for dt in range(kd)]
                    for dt in range(kd):
                        for m in range(kd):
                            nc.tensor.matmul(
                                py[dt][:], lhsT=w_t[1][m][dt][:],
                                rhs=rT[:, m, 0:ns],
                                start=(m == 0), stop=(m == kd - 1))
                    yT = zpool.tile([P, kd, cfg.dense_n], f32, tag="yT")
                    for dt in range(kd):
                        nc.scalar.activation(
                            yT[:, dt, 0:ns], py[dt][:],
                            mybir.ActivationFunctionType.Identity,
                            bias=bias_t[1][:, dt:dt + 1], scale=1.0)
                    final = yT

                for jj in range(ns // P):
                    blk = (s0 + jj * P) // P
                    ost = stpool.tile([P, d], bf16, tag="ost")
                    for dt in range(kd):
                        ptr2 = trpsum.tile([P, P], f32, space="PSUM",
                                           tag="ptr")
                        nc.tensor.transpose(
                            out=ptr2[:],
                            in_=final[:, dt, jj * P:(jj + 1) * P],
                            identity=ident[:])
                        nc.vector.tensor_copy(
                            ost[:, dt * P:(dt + 1) * P], ptr2[:])
                    nc.sync.dma_start(out_d[blk * P:(blk + 1) * P, :], ost[:])

            # ---- aggregation pass with dense slices interleaved (LAG
            # blocks after a slice's last aggregation block)
            LAG = 2
            qload = [0] * NSWQ  # greedy row-balance across SWDGE queues
            ci = [0, 0]  # per-group next-call cursor (A calls precede B)
            ncallA = len([c for c in calls if c[0] == 0])
            gtiles = [[], []]  # per-group list of emitted gt tile handles

            def ensure_call(grp, t):
                # emit group-grp gather calls (in order) until tile t is
                # covered; consumption is sequential so this emits <= 1
                # call per steady-state invocation
                base = 0 if grp == 0 else ncallA
                ncg = ncallA if grp == 0 else len(calls) - ncallA
                while ci[grp] < ncg:
                    _, t0, cn, ic = calls[base + ci[grp]]
                    if t < t0:
                        return
                    pool_g = gApool if grp == 0 else gBpool
                    tab_ap = tablo if grp == 0 else tabhi
                    gtag = "gA" if grp == 0 else "gB"
                    gt = pool_g.tile([P, GCH, d], gdt, tag=gtag,
                                     name=f"g_{gtag}_{ci[grp]}")
                    q = qload.index(min(qload))
                    nc.gpsimd.dma_gather(
                        out_ap=gt[:, 0:cn, :], in_ap=tab_ap[:],
                        idxs_ap=idxp_t[:, ic:ic + cn * 8],
                        num_idxs=cn * P, num_idxs_reg=cn * P, elem_size=d,
                        queue_num=q)
                    qload[q] += cn
                    gtiles[grp].append(gt)
                    ci[grp] += 1
                    if t < t0 + cn:
                        return

            next_s = 0
            for b in range(nb):
                ta, tb = int(TA[b]), int(TB[b])
                tbt = ta + tb
                tA0, tB0 = int(ta_base[b]), int(tb_base[b])
                psum_a = apsum.tile([P, d], f32, space="PSUM", tag="psum_a")
                if tbt:
                    p_all = ppool.tile([P, tmaxP, P], gdt, tag="pmat")
                    if ta:
                        nc.vector.tensor_tensor(
                            out=p_all[:, 0:ta, :],
                            in0=slotp_t[:, tA0:tA0 + ta, None].to_broadcast(
                                [P, ta, P]),
                            in1=iota_f[:, None, :].to_broadcast([P, ta, P]),
                            op=mybir.AluOpType.is_equal)
                    if tb:
                        nc.vector.tensor_tensor(
                            out=p_all[:, ta:tbt, :],
                            in0=slotp_t[:, tB0:tB0 + tb, None].to_broadcast(
                                [P, tb, P]),
                            in1=iota_f[:, None, :].to_broadcast([P, tb, P]),
                            op=mybir.AluOpType.is_equal)
                selft = locpool.tile([P, d], gdt, tag="selft")
                nc.sync.dma_start(selft[:], loctab[b * P:(b + 1) * P, :])
                nc.tensor.matmul(psum_a[:], lhsT=ident_g[:], rhs=selft[:],
                                 start=True, stop=(tbt == 0))
                for j in range(tbt):
                    grp = 0 if j < ta else 1
                    t = tA0 + j if grp == 0 else tB0 + (j - ta)
                    ensure_call(grp, t)
                    off = 0 if grp == 0 else totA
                    gt = gtiles[grp][(t - off) // GCH]
                    nc.tensor.matmul(
                        psum_a[:], lhsT=p_all[:, j, :],
                        rhs=gt[:, (t - off) % GCH, :],
                        start=False, stop=(j == tbt - 1))

                # epilogue: u' = psum * dinv (self-loops are gathered edges)
                u2 = wpool.tile([P, d], f32, tag="u2")
                nc.scalar.mul(u2[:], psum_a[:], dinvw_t[:, b:b + 1])
                sl, off = divmod(b * P, cfg.dense_n)
                for m in range(kd):
                    ptr = trpsum.tile([P, P], f32, space="PSUM", tag="ptr")
                    nc.tensor.transpose(out=ptr[:],
                                        in_=u2[:, m * P:(m + 1) * P],
                                        identity=ident[:])
                    nc.vector.tensor_copy(uT_s[sl][:, m, off:off + P], ptr[:])

                if b == 0:
                    load_weights()
                while next_s < nsl and min(4 * next_s + 3, nb - 1) <= b - LAG:
                    emit_dense(next_s)
                    next_s += 1

            while next_s < nsl:
                emit_dense(next_s)
                next_s += 1

    nc.compile()
    return nc


# ------------------------------------------------------------------ driver


def _run(cfg, nc_prog, per_core_common, per_core_vars, trace=False):
    in_maps = []
    for c in range(cfg.nc):
        m = dict(per_core_common)
        m.update(per_core_vars[c])
        in_maps.append(m)
    res = run_bass_kernel_spmd(nc_prog, in_maps, core_ids=list(range(cfg.nc)),
                               trace=trace)
    return res


def gcn_forward(cfg, x, edge_index, W_pre, b_pre, W1, b1, W2, b2, W_post,
                b_post, trace=False, ret_times=None):
    x = np.asarray(x, np.float32)
    src = np.asarray(edge_index[0], np.int64)
    dst = np.asarray(edge_index[1], np.int64)
    W_pre, W1, W2, W_post = (np.asarray(w, np.float32)
                             for w in (W_pre, W1, W2, W_post))
    b_pre, b1, b2, b_post = (np.asarray(b, np.float32)
                             for b in (b_pre, b1, b2, b_post))

    n, d, nl, nb, npad = cfg.n_nodes, cfg.d, cfg.nloc, cfg.nblk, cfg.npad
    deg = (np.bincount(dst, minlength=n) + 1).astype(np.float64)
    dinv = (1.0 / np.sqrt(deg)).astype(np.float32)

    TA, TB, edge_planes = _prep_edges(cfg, src, dst)

    def local_pad(tab, c):
        out = np.zeros((npad, d), tab.dtype)
        out[:nl] = tab[c * nl:(c + 1) * nl]
        return out

    xs = x * dinv[:, None]
    WA = (W_pre.astype(np.float64) @ W1.astype(np.float64)).astype(np.float32)

    has_bpre = bool(np.any(b_pre != 0))
    dinv_cols = [
        _wrap_cols(dinv[c * nl:(c + 1) * nl], nb, npad) for c in range(cfg.nc)]

    # ---------- launch 1
    prog1 = build_launch(cfg, 1, TA, TB, has_bpre=has_bpre)
    tdt = ml_dtypes.bfloat16 if SCAT_BF16 else np.float32
    common1 = {
        "tablo": xs[: cfg.split].astype(tdt),
        "tabhi": xs[cfg.n_nodes - cfg.nhi:].astype(tdt),
        "w0": WA,
        "bias0": b1.reshape(d // P, P).T.copy(),
    }
    if has_bpre:
        v1 = (b_pre.astype(np.float64) @ W1.astype(np.float64)).astype(
            np.float32)
        common1["v1w"] = v1.reshape(d // P, P).T.copy()
        # c1[dst] = (s[dst] + dinv[dst]) * dinv[dst],  s = sum_e dinv[src]
        s = np.zeros(n, np.float64)
        np.add.at(s, dst, dinv[src].astype(np.float64))
        c1_full = ((s + dinv) * dinv).astype(np.float32)
    vars1 = []
    for c in range(cfg.nc):
        v = {
            "loctab": local_pad(xs.astype(tdt), c),
            "idxp": edge_planes[c]["idxp"],
            "slotp": edge_planes[c]["slotp"],
            "dinvw": dinv_cols[c],
        }
        if has_bpre:
            cl = np.zeros(npad, np.float32)
            cl[:nl] = c1_full[c * nl:(c + 1) * nl]
            v["c1rep"] = np.tile(cl, (P, 1))
        vars1.append(v)
    res1 = _run(cfg, prog1, common1, vars1, trace=trace)
    g1 = np.concatenate([res1.results[c]["out"][:nl] for c in range(cfg.nc)]
                        ).astype(np.float32)
    g1 *= dinv[:, None]
    if ret_times is not None:
        ret_times.append(res1.exec_time_ns)

    # ---------- launch 2
    prog2 = build_launch(cfg, 2, TA, TB, has_bpre=False)
    common2 = {
        "tablo": g1[: cfg.split].astype(tdt),
        "tabhi": g1[cfg.n_nodes - cfg.nhi:].astype(tdt),
        "w0": W2,
        "w1": W_post,
        "bias0": b2.reshape(d // P, P).T.copy(),
        "bias1": b_post.reshape(d // P, P).T.copy(),
    }
    vars2 = []
    for c in range(cfg.nc):
        vars2.append({
            "loctab": local_pad(g1.astype(tdt), c),
            "idxp": edge_planes[c]["idxp"],
            "slotp": edge_planes[c]["slotp"],
            "dinvw": dinv_cols[c],
        })
    res2 = _run(cfg, prog2, common2, vars2, trace=trace)
    y = np.concatenate([res2.results[c]["out"][:nl] for c in range(cfg.nc)]
                       ).astype(np.float32)
    if ret_times is not None:
        ret_times.append(res2.exec_time_ns)
    return y


def kernel(x, edge_index, W_pre, b_pre, W1, b1, W2, b2, W_post, b_post):
    cfg = Cfg()
    return gcn_forward(cfg, x, edge_index, W_pre, b_pre, W1, b1, W2, b2,
                       W_post, b_post)



# revision 9
# speedup vs baseline: 1.0726x; 1.0726x over previous
"""GCN NodeAttributeAggregator on 8 Trainium2 NeuronCores.

Strategy (node-sharded, dst-partitioned edges):
  - Host precomputes index metadata: degrees (with self-loops), dinv=rsqrt(deg),
    per-core edge lists partitioned by dst owner, grouped by (dst-block of 128,
    src<32768 vs src>=32768 for int16 gather indices), padded to 128-edge tiles.
  - Device per core: dma_gather rows of a pre-scaled node table (xs = x*dinv),
    scatter-by-matmul: one-hot P matrices (built on DVE via iota + is_equal)
    contract each 128-edge tile into a 128-dst PSUM accumulator.
  - Gather tables are fp8 e3m4 (256B rows — half the HBM/descriptor bytes of
    bf16); the power-of-2 quantization scale folds into the dinv epilogue.
  - Dense 256x256 matmuls run in bf16 feature-major (FWL full rate), with
    bf16 PE transposes at layout boundaries; outputs written feature-major
    in large contiguous DMAs (host untransposes).
  - Algebra: GCN layer out = D^-1/2 (A+I) D^-1/2 h W.  Row scaling commutes
    with right matmuls, relu commutes with positive row scaling, and
    agg(h W) = agg(h) W, so:
      L1 (fused W_pre@W1): u' = (scatter(xs) + xs_dst) * dinv/S1;
          g1 = relu(u' @ (W_pre W1) + b1 + rank1(b_pre))
      L2: v' = (scatter(q(g1*dinv)) + ...) * dinv/S2;
          y  = relu(v' @ W2 + b2) @ W_post + b_post
  - Two SPMD launches; host rescales/requantizes g1 between them.
"""

import dataclasses
import ml_dtypes
import numpy as np

import concourse.bacc as bacc
import concourse.bass as bass
import concourse.tile as tile
import concourse.mybir as mybir
from concourse.bass_utils import run_bass_kernel_spmd
from concourse.masks import make_identity

P = 128
NSWQ = 4  # SWDGE queues
GCH = 8  # gather tiles per dma_gather call
f32 = mybir.dt.float32
bf16 = mybir.dt.bfloat16
fp8 = mybir.dt.float8e3  # e3m4
FP8_TAB = True  # gather tables in e3m4 (half bytes) vs bf16
gdt = fp8 if FP8_TAB else bf16
np_gdt = ml_dtypes.float8_e3m4 if FP8_TAB else ml_dtypes.bfloat16


def _qscale(absmax):
    """Power-of-2 scale putting absmax at ~12 (e3m4 max finite = 15.5)."""
    return float(2.0 ** np.floor(np.log2(12.0 / max(absmax, 1e-30))))
i16 = mybir.dt.int16
i32 = mybir.dt.int32


@dataclasses.dataclass
class Cfg:
    n_nodes: int = 50000
    d: int = 256
    nc: int = 8
    split: int = 32768
    dense_n: int = 512

    @property
    def nloc(self):
        return self.n_nodes // self.nc

    @property
    def nblk(self):
        return (self.nloc + P - 1) // P

    @property
    def npad(self):
        return self.nblk * P

    @property
    def nhi(self):
        # table B spans the last min(32768, n) rows (full int16 window)
        return min(self.split, self.n_nodes)


# ---------------------------------------------------------------- host prep


def _wrap16(vals):
    """[n] -> [128, n//16] int16: value i at [i%16, i//16], replicated x8."""
    w = vals.reshape(-1, 16).T
    return np.tile(w, (8, 1)).astype(np.int16)


def _prep_edges(cfg, src, dst):
    """Partition edges by dst owner; split per dst-block into two gather
    groups (table A = rows [0, split); table B = rows [baseB, n)).  The cut
    is chosen per block at an exact multiple of 128 edges in src-order when
    the overlap window [baseB, split) allows, so group A has no padding.

    Returns (TA, TB, per-core dict of idx/slot planes) with identical
    compile-time schedule (TA, TB) across cores.
    """
    nl, nb = cfg.nloc, cfg.nblk
    baseB = cfg.n_nodes - cfg.nhi
    owner = dst // nl
    loc = dst - owner * nl
    blk = loc // P
    slot = loc - blk * P

    key = owner * nb + blk
    nkeys = cfg.nc * nb
    n_cb = np.bincount(key, minlength=nkeys).reshape(cfg.nc, nb)
    cntlo = np.bincount(key[src < baseB], minlength=nkeys).reshape(cfg.nc, nb)
    cntA = np.bincount(key[src < cfg.split], minlength=nkeys).reshape(cfg.nc, nb)

    lo = -(-cntlo // P).max(axis=0)          # [nb] min feasible TA
    hi = (cntA // P).min(axis=0)             # [nb] max feasible TA
    TA = np.maximum(hi, lo)                  # prefer max A (A has no pad)
    feasible = lo <= hi
    # fallback for infeasible blocks: threshold split at `split`
    TA = np.where(feasible, TA, -(-cntA // P).max(axis=0))
    # B count per (c,b): n - A_count; A_count = TA*P if feasible else cntA
    A_cnt = np.where(feasible[None, :], np.minimum(TA[None, :] * P, n_cb),
                     cntA)
    B_cnt = n_cb - A_cnt
    TB = (-(-B_cnt // P)).max(axis=0)

    # global A/B layout: all blocks' A segments first, then all B segments,
    # so gather calls can span block boundaries within one table
    baseA = np.concatenate([[0], np.cumsum(TA * P)])[:-1]
    baseBs = int(TA.sum()) * P + np.concatenate([[0], np.cumsum(TB * P)])[:-1]
    gbase = np.stack([baseA, baseBs], axis=1).astype(np.int64)
    rows_total = int((TA.sum() + TB.sum()) * P)

    # rank edges within (core, block) by src (stable) to apply the cut
    order = np.lexsort((src, key))
    skey = key[order]
    group_start = np.concatenate(
        [[0], np.cumsum(np.bincount(skey, minlength=nkeys))])
    rank = np.arange(len(src)) - group_start[skey]

    ocore = owner[order]
    oblk = blk[order]
    acut = A_cnt[ocore, oblk]
    in_a = rank < acut
    grp = (~in_a).astype(np.int64)
    rowpos = np.where(in_a, rank, rank - acut)
    rows = gbase[oblk, grp] + rowpos
    idxv = (src[order] - grp * baseB).astype(np.int16)
    slotv = slot[order].astype(np.float32)
    assert (src[order][in_a] < cfg.split).all()
    assert (src[order][~in_a] >= baseB).all()

    idx_flat = np.zeros((cfg.nc, rows_total), np.int16)
    slot_flat = np.full((cfg.nc, rows_total), 300.0, np.float32)
    idx_flat[ocore, rows] = idxv
    slot_flat[ocore, rows] = slotv

    per_core = []
    for c in range(cfg.nc):
        idxp = _wrap16(idx_flat[c])
        slotp = slot_flat[c].reshape(-1, P).T.copy()
        per_core.append({"idxp": idxp, "slotp": slotp})
    return TA, TB, per_core


def _wrap_cols(vec, nblk, npad):
    """[npad] -> [128, nblk] with [p, b] = vec[b*128+p]."""
    v = np.zeros(npad, np.float32)
    v[: len(vec)] = vec
    return v.reshape(nblk, P).T.copy()


def _wrap_tab(tab, nblk, d):
    """[npad, d] -> [128, nblk*d] with [p, b*d:(b+1)*d] = tab[b*128+p]."""
    return tab.reshape(nblk, P, d).transpose(1, 0, 2).reshape(P, nblk * d)


# ------------------------------------------------------------- device build


def build_launch(cfg, mode, TA, TB, has_bpre=False):
    """mode 1: out = relu(u' @ WA + b1 [+ rank1])   (writes g1, feat-major)
    mode 2: out = relu(v' @ W2 + b2) @ W_post + b_post  (writes y, feat-major)
    """
    nb, npad, d = cfg.nblk, cfg.npad, cfg.d
    ntiles = int((TA + TB).sum())
    nidxcol = ntiles * 8
    tmaxP = max(int((TA + TB).max()), 1)

    # global A/B tile layout (matches host): A tiles of all blocks first,
    # then all B tiles.  Gather calls span block boundaries, so nearly all
    # calls are full GCH tiles.
    totA = int(TA.sum())
    ta_base = np.concatenate([[0], np.cumsum(TA)])[:-1]
    tb_base = totA + np.concatenate([[0], np.cumsum(TB)])[:-1]

    # gather-call plan per group: (grp, t0, cn, icol); icol = t0 * 8
    calls = []
    for grp, gt0, gcnt in ((0, 0, totA), (1, totA, int(TB.sum()))):
        for c0 in range(0, gcnt, GCH):
            cn = min(GCH, gcnt - c0)
            calls.append((grp, gt0 + c0, cn, (gt0 + c0) * 8))
    assert sum(c[2] for c in calls) == ntiles

    # split the idx plane into chunks at call boundaries so early gathers
    # only wait on the first chunks; A and B ranges are chunked separately
    # and their DMAs interleaved (block 0 needs the head of both ranges)
    acol = totA * 8
    astarts = [c[3] for c in calls if c[0] == 0]
    bstarts = [c[3] for c in calls if c[0] == 1]

    def _cuts(starts, lo, hi, fracs):
        cs = [lo]
        for fr in fracs:
            tgt = lo + (hi - lo) * fr
            best = min(starts, key=lambda x: abs(x - tgt))
            if cs[-1] < best < hi:
                cs.append(best)
        cs.append(hi)
        return list(zip(cs[:-1], cs[1:]))

    a_ch = _cuts(astarts, 0, acol, [0.02, 0.10, 0.30, 0.60])
    b_ch = (_cuts(bstarts, acol, nidxcol, [0.04, 0.50])
            if acol < nidxcol else [])
    chunks = []
    for i in range(max(len(a_ch), len(b_ch))):
        if i < len(a_ch):
            chunks.append(a_ch[i])
        if i < len(b_ch):
            chunks.append(b_ch[i])

    nc = bacc.Bacc("TRN2", target_bir_lowering=False, debug=False,
                   num_devices=cfg.nc, num_swdge_queues=NSWQ)

    tablo = nc.dram_tensor("tablo", [cfg.split, d], gdt, kind="ExternalInput")
    loctab = nc.dram_tensor("loctab", [P, nb * d], gdt, kind="ExternalInput")
    tabhi = nc.dram_tensor("tabhi", [cfg.nhi, d], gdt, kind="ExternalInput")
    idxp_d = nc.dram_tensor("idxp", [P, nidxcol], i16, kind="ExternalInput")
    slotp_d = nc.dram_tensor("slotp", [P, ntiles], f32, kind="ExternalInput")
    dinvw_d = nc.dram_tensor("dinvw", [P, nb], f32, kind="ExternalInput")
    nw = 1 if mode == 1 else 2
    w_d = [nc.dram_tensor(f"w{i}", [d, d], bf16, kind="ExternalInput")
           for i in range(nw)]
    bias_d = [nc.dram_tensor(f"bias{i}", [P, d // P], f32, kind="ExternalInput")
              for i in range(nw)]
    if has_bpre:
        c1rep_d = nc.dram_tensor("c1rep", [P, npad], f32, kind="ExternalInput")
        v1w_d = nc.dram_tensor("v1w", [P, d // P], f32, kind="ExternalInput")
    kd = d // P  # feature k-tiles (2)
    out_d = nc.dram_tensor("out", [kd, P, npad], bf16, kind="ExternalOutput")

    with tile.TileContext(nc) as tc:
        with (
            tc.tile_pool(name="const", bufs=1) as cpool,
            tc.tile_pool(name="gA", bufs=8) as gApool,
            tc.tile_pool(name="gB", bufs=6) as gBpool,
            tc.tile_pool(name="pmat", bufs=3) as ppool,
            tc.tile_pool(name="work", bufs=3) as wpool,
            tc.tile_pool(name="zslab", bufs=2) as zpool,
            tc.tile_pool(name="apsum", bufs=3, space="PSUM") as apsum,
            tc.tile_pool(name="trpsum", bufs=2, space="PSUM") as trpsum,
            tc.tile_pool(name="dpsum", bufs=3, space="PSUM") as dpsum,
        ):
            # ---- warmup: tiny gathers on all queues absorb SWDGE ucode
            # cold-start while the idx planes stream in
            izero = cpool.tile([P, 8], i16)
            nc.gpsimd.memset(izero[:], 0)
            warm = cpool.tile([P, NSWQ, d], gdt)
            for q in range(NSWQ):
                nc.gpsimd.dma_gather(
                    out_ap=warm[:, q:q + 1, :], in_ap=tablo[:],
                    idxs_ap=izero[:],
                    num_idxs=P, num_idxs_reg=P, elem_size=d, queue_num=q)

            # ---- constants (idx planes first: gathers depend on them);
            # slotp/dinvw go right after the first small idx chunks so
            # block 0's P-matrix build isn't starved
            idxp_t = cpool.tile([P, nidxcol], i16)
            slotp_t = cpool.tile([P, ntiles], f32)
            dinvw_t = cpool.tile([P, nb], f32)
            for ci_, (lo, hi) in enumerate(chunks):
                nc.sync.dma_start(idxp_t[:, lo:hi], idxp_d[:, lo:hi])
                if ci_ == min(1, len(chunks) - 1):
                    nc.sync.dma_start(slotp_t[:], slotp_d[:])
                    nc.sync.dma_start(dinvw_t[:], dinvw_d[:])
            # local (self-loop) table: prewrapped, big contiguous loads on
            # the scalar HWDGE queue (parallel with idx loads on sync)
            loctab_t = cpool.tile([P, nb, d], gdt)
            lsplit = min(6, nb)
            nc.scalar.dma_start(loctab_t[:, 0:lsplit, :],
                                loctab[:, 0:lsplit * d])
            if lsplit < nb:
                nc.scalar.dma_start(loctab_t[:, lsplit:nb, :],
                                    loctab[:, lsplit * d:nb * d])
            iota_i = cpool.tile([P, P], i32)
            nc.gpsimd.iota(iota_i[:], pattern=[[1, P]], base=0,
                           channel_multiplier=0)
            iota_f = cpool.tile([P, P], f32)
            nc.vector.tensor_copy(iota_f[:], iota_i[:])
            ident = cpool.tile([P, P], f32)
            make_identity(nc, ident[:])
            ident_b = cpool.tile([P, P], bf16)
            nc.vector.tensor_copy(ident_b[:], ident[:])
            if has_bpre:
                c1rep_t = cpool.tile([P, npad], f32)
                nc.sync.dma_start(c1rep_t[:], c1rep_d[:])
                v1w_t = cpool.tile([P, kd], f32)
                nc.sync.dma_start(v1w_t[:], v1w_d[:])

            # weights/biases are loaded lazily (after block 0's gathers are
            # queued) on the scalar queue
            w_t = [[[None] * kd for _ in range(kd)] for _ in range(nw)]
            bias_t = [None] * nw

            def load_weights():
                for i in range(nw):
                    for k in range(kd):
                        for m in range(kd):
                            wt = cpool.tile([P, P], bf16,
                                            name=f"wt{i}_{k}_{m}",
                                            tag=f"wt{i}_{k}_{m}")
                            nc.scalar.dma_start(
                                wt[:],
                                w_d[i][k * P:(k + 1) * P, m * P:(m + 1) * P])
                            w_t[i][k][m] = wt
                    bt = cpool.tile([P, kd], f32, name=f"bt{i}", tag=f"bt{i}")
                    nc.scalar.dma_start(bt[:], bias_d[i][:])
                    bias_t[i] = bt

            # feature-major activations, one tile per dense node-slice
            nsl = (npad + cfg.dense_n - 1) // cfg.dense_n
            uT_s = [cpool.tile([P, kd, min(cfg.dense_n, npad - i * cfg.dense_n)],
                               bf16, name=f"uTs{i}", tag=f"uTs{i}")
                    for i in range(nsl)]

            def emit_dense(s):
                s0 = s * cfg.dense_n
                ns = min(cfg.dense_n, npad - s0)
                pz = [dpsum.tile([P, ns], f32, space="PSUM", tag="dps",
                                 name=f"pz{s}_{dt}") for dt in range(kd)]
                for dt in range(kd):
                    for m in range(kd):
                        nc.tensor.matmul(
                            pz[dt][:], lhsT=w_t[0][m][dt][:],
                            rhs=uT_s[s][:, m, 0:ns],
                            start=(m == 0), stop=(m == kd - 1))
                if has_bpre:
                    for dt in range(kd):
                        tmp = wpool.tile([P, cfg.dense_n], f32, tag="r1")
                        nc.vector.tensor_scalar_mul(
                            tmp[:, 0:ns], c1rep_t[:, s0:s0 + ns],
                            v1w_t[:, dt:dt + 1])
                        nc.vector.tensor_tensor(
                            out=pz[dt][:], in0=pz[dt][:], in1=tmp[:, 0:ns],
                            op=mybir.AluOpType.add)

                if mode == 1:
                    final = zpool.tile([P, kd, cfg.dense_n], bf16, tag="zr")
                    for dt in range(kd):
                        nc.scalar.activation(
                            final[:, dt, 0:ns], pz[dt][:],
                            mybir.ActivationFunctionType.Relu,
                            bias=bias_t[0][:, dt:dt + 1], scale=1.0)
                else:
                    rT = zpool.tile([P, kd, cfg.dense_n], bf16, tag="zr")
                    for dt in range(kd):
                        nc.scalar.activation(
                            rT[:, dt, 0:ns], pz[dt][:],
                            mybir.ActivationFunctionType.Relu,
                            bias=bias_t[0][:, dt:dt + 1], scale=1.0)
                    py = [dpsum.tile([P, ns], f32, space="PSUM", tag="dps",
                                     name=f"py{s}_{dt}") for dt in range(kd)]
                    for dt in range(kd):
                        for m in range(kd):
                            nc.tensor.matmul(
                                py[dt][:], lhsT=w_t[1][m][dt][:],
                                rhs=rT[:, m, 0:ns],
                                start=(m == 0), stop=(m == kd - 1))
                    final = zpool.tile([P, kd, cfg.dense_n], bf16, tag="yT")
                    for dt in range(kd):
                        nc.scalar.activation(
                            final[:, dt, 0:ns], py[dt][:],
                            mybir.ActivationFunctionType.Identity,
                            bias=bias_t[1][:, dt:dt + 1], scale=1.0)

                for dt in range(kd):
                    nc.scalar.dma_start(out_d[dt, :, s0:s0 + ns],
                                        final[:, dt, 0:ns])

            # ---- aggregation pass with dense slices interleaved (LAG
            # blocks after a slice's last aggregation block)
            LAG = 2
            qload = [0] * NSWQ  # greedy row-balance across SWDGE queues
            ci = [0, 0]  # per-group next-call cursor (A calls precede B)
            ncallA = len([c for c in calls if c[0] == 0])
            gtiles = [[], []]  # per-group list of emitted gt tile handles

            def ensure_call(grp, t):
                # emit group-grp gather calls (in order) until tile t is
                # covered; consumption is sequential so this emits <= 1
                # call per steady-state invocation
                base = 0 if grp == 0 else ncallA
                ncg = ncallA if grp == 0 else len(calls) - ncallA
                while ci[grp] < ncg:
                    _, t0, cn, ic = calls[base + ci[grp]]
                    if t < t0:
                        return
                    pool_g = gApool if grp == 0 else gBpool
                    tab_ap = tablo if grp == 0 else tabhi
                    gtag = "gA" if grp == 0 else "gB"
                    gt = pool_g.tile([P, GCH, d], gdt, tag=gtag,
                                     name=f"g_{gtag}_{ci[grp]}")
                    q = qload.index(min(qload))
                    nc.gpsimd.dma_gather(
                        out_ap=gt[:, 0:cn, :], in_ap=tab_ap[:],
                        idxs_ap=idxp_t[:, ic:ic + cn * 8],
                        num_idxs=cn * P, num_idxs_reg=cn * P, elem_size=d,
                        queue_num=q)
                    qload[q] += cn
                    gtiles[grp].append(gt)
                    ci[grp] += 1
                    if t < t0 + cn:
                        return

            next_s = 0
            for b in range(nb):
                ta, tb = int(TA[b]), int(TB[b])
                tbt = ta + tb
                tA0, tB0 = int(ta_base[b]), int(tb_base[b])
                psum_a = apsum.tile([P, d], f32, space="PSUM", tag="psum_a")
                if tbt:
                    p_all = ppool.tile([P, tmaxP, P], bf16, tag="pmat")
                    if ta:
                        nc.vector.tensor_tensor(
                            out=p_all[:, 0:ta, :],
                            in0=slotp_t[:, tA0:tA0 + ta, None].to_broadcast(
                                [P, ta, P]),
                            in1=iota_f[:, None, :].to_broadcast([P, ta, P]),
                            op=mybir.AluOpType.is_equal)
                    if tb:
                        nc.vector.tensor_tensor(
                            out=p_all[:, ta:tbt, :],
                            in0=slotp_t[:, tB0:tB0 + tb, None].to_broadcast(
                                [P, tb, P]),
                            in1=iota_f[:, None, :].to_broadcast([P, tb, P]),
                            op=mybir.AluOpType.is_equal)
                nc.tensor.matmul(psum_a[:], lhsT=ident_b[:],
                                 rhs=loctab_t[:, b, :],
                                 start=True, stop=(tbt == 0))
                for j in range(tbt):
                    grp = 0 if j < ta else 1
                    t = tA0 + j if grp == 0 else tB0 + (j - ta)
                    ensure_call(grp, t)
                    off = 0 if grp == 0 else totA
                    gt = gtiles[grp][(t - off) // GCH]
                    nc.tensor.matmul(
                        psum_a[:], lhsT=p_all[:, j, :],
                        rhs=gt[:, (t - off) % GCH, :],
                        start=False, stop=(j == tbt - 1))

                # epilogue: u' = psum * dinv/S (self-loops are gathered rows)
                u2 = wpool.tile([P, d], bf16, tag="u2")
                nc.scalar.mul(u2[:], psum_a[:], dinvw_t[:, b:b + 1])
                sl, off = divmod(b * P, cfg.dense_n)
                for m in range(kd):
                    ptr = trpsum.tile([P, P], bf16, space="PSUM", tag="ptr")
                    nc.tensor.transpose(out=ptr[:],
                                        in_=u2[:, m * P:(m + 1) * P],
                                        identity=ident_b[:])
                    nc.vector.tensor_copy(uT_s[sl][:, m, off:off + P], ptr[:])

                if b == 0:
                    load_weights()
                while next_s < nsl and min(4 * next_s + 3, nb - 1) <= b - LAG:
                    emit_dense(next_s)
                    next_s += 1

            while next_s < nsl:
                emit_dense(next_s)
                next_s += 1

    nc.compile()
    return nc


# ------------------------------------------------------------------ driver


def _run(cfg, nc_prog, per_core_common, per_core_vars, trace=False):
    in_maps = []
    for c in range(cfg.nc):
        m = dict(per_core_common)
        m.update(per_core_vars[c])
        in_maps.append(m)
    res = run_bass_kernel_spmd(nc_prog, in_maps, core_ids=list(range(cfg.nc)),
                               trace=trace)
    return res


def gcn_forward(cfg, x, edge_index, W_pre, b_pre, W1, b1, W2, b2, W_post,
                b_post, trace=False, ret_times=None):
    x = np.asarray(x, np.float32)
    src = np.asarray(edge_index[0], np.int64)
    dst = np.asarray(edge_index[1], np.int64)
    W_pre, W1, W2, W_post = (np.asarray(w, np.float32)
                             for w in (W_pre, W1, W2, W_post))
    b_pre, b1, b2, b_post = (np.asarray(b, np.float32)
                             for b in (b_pre, b1, b2, b_post))

    n, d, nl, nb, npad = cfg.n_nodes, cfg.d, cfg.nloc, cfg.nblk, cfg.npad
    deg = (np.bincount(dst, minlength=n) + 1).astype(np.float64)
    dinv = (1.0 / np.sqrt(deg)).astype(np.float32)

    TA, TB, edge_planes = _prep_edges(cfg, src, dst)

    def local_pad(tab, c):
        out = np.zeros((npad, d), tab.dtype)
        out[:nl] = tab[c * nl:(c + 1) * nl]
        return _wrap_tab(out, nb, d)

    xs = x * dinv[:, None]
    WA = (W_pre.astype(np.float64) @ W1.astype(np.float64)).astype(np.float32)

    has_bpre = bool(np.any(b_pre != 0))

    def unpack_out(res, c):
        # [kd, P, npad] feature-major -> [nl, d]
        o = np.asarray(res.results[c]["out"])
        return o.reshape(d, npad).T[:nl]

    # ---------- launch 1
    prog1 = build_launch(cfg, 1, TA, TB, has_bpre=has_bpre)
    S1 = _qscale(float(np.abs(xs).max()))
    xq = (xs * np.float32(S1)).astype(np_gdt)
    dinv1 = (dinv / np.float32(S1)).astype(np.float32)
    common1 = {
        "tablo": xq[: cfg.split],
        "tabhi": xq[cfg.n_nodes - cfg.nhi:],
        "w0": WA.astype(ml_dtypes.bfloat16),
        "bias0": b1.reshape(d // P, P).T.copy(),
    }
    if has_bpre:
        v1 = (b_pre.astype(np.float64) @ W1.astype(np.float64)).astype(
            np.float32)
        common1["v1w"] = v1.reshape(d // P, P).T.copy()
        # c1[dst] = (s[dst] + dinv[dst]) * dinv[dst],  s = sum_e dinv[src]
        s = np.zeros(n, np.float64)
        np.add.at(s, dst, dinv[src].astype(np.float64))
        c1_full = ((s + dinv) * dinv).astype(np.float32)
    vars1 = []
    for c in range(cfg.nc):
        v = {
            "loctab": local_pad(xq, c),
            "idxp": edge_planes[c]["idxp"],
            "slotp": edge_planes[c]["slotp"],
            "dinvw": _wrap_cols(dinv1[c * nl:(c + 1) * nl], nb, npad),
        }
        if has_bpre:
            cl = np.zeros(npad, np.float32)
            cl[:nl] = c1_full[c * nl:(c + 1) * nl]
            v["c1rep"] = np.tile(cl, (P, 1))
        vars1.append(v)
    res1 = _run(cfg, prog1, common1, vars1, trace=trace)
    g1 = np.concatenate([unpack_out(res1, c) for c in range(cfg.nc)]
                        ).astype(np.float32)
    g1 *= dinv[:, None]
    if ret_times is not None:
        ret_times.append(res1.exec_time_ns)

    # ---------- launch 2
    prog2 = build_launch(cfg, 2, TA, TB, has_bpre=False)
    S2 = _qscale(float(np.abs(g1).max()))
    g1q = (g1 * np.float32(S2)).astype(np_gdt)
    dinv2 = (dinv / np.float32(S2)).astype(np.float32)
    common2 = {
        "tablo": g1q[: cfg.split],
        "tabhi": g1q[cfg.n_nodes - cfg.nhi:],
        "w0": W2.astype(ml_dtypes.bfloat16),
        "w1": W_post.astype(ml_dtypes.bfloat16),
        "bias0": b2.reshape(d // P, P).T.copy(),
        "bias1": b_post.reshape(d // P, P).T.copy(),
    }
    vars2 = []
    for c in range(cfg.nc):
        vars2.append({
            "loctab": local_pad(g1q, c),
            "idxp": edge_planes[c]["idxp"],
            "slotp": edge_planes[c]["slotp"],
            "dinvw": _wrap_cols(dinv2[c * nl:(c + 1) * nl], nb, npad),
        })
    res2 = _run(cfg, prog2, common2, vars2, trace=trace)
    y = np.concatenate([unpack_out(res2, c) for c in range(cfg.nc)]
                       ).astype(np.float32)
    if ret_times is not None:
        ret_times.append(res2.exec_time_ns)
    return y


def kernel(x, edge_index, W_pre, b_pre, W1, b1, W2, b2, W_post, b_post):
    cfg = Cfg()
    return gcn_forward(cfg, x, edge_index, W_pre, b_pre, W1, b1, W2, b2,
                       W_post, b_post)


# revision 26
# speedup vs baseline: 1.0955x; 1.0214x over previous
"""GCN NodeAttributeAggregator on 8 Trainium2 NeuronCores.

Strategy (node-sharded, dst-partitioned edges):
  - Host precomputes index metadata: degrees (with self-loops), dinv=rsqrt(deg),
    per-core edge lists partitioned by dst owner, grouped by (dst-block of 128,
    src<32768 vs src>=32768 for int16 gather indices), padded to 128-edge tiles.
  - Device per core: dma_gather rows of a pre-scaled node table (xs = x*dinv),
    scatter-by-matmul: one-hot P matrices (built on DVE via iota + is_equal)
    contract each 128-edge tile into a 128-dst PSUM accumulator.
  - Gather tables are fp8 e3m4 (256B rows — half the HBM/descriptor bytes of
    bf16); the power-of-2 quantization scale folds into the dinv epilogue.
  - Dense 256x256 matmuls run in bf16 feature-major (FWL full rate), with
    bf16 PE transposes at layout boundaries; outputs written feature-major
    in large contiguous DMAs (host untransposes).
  - Algebra: GCN layer out = D^-1/2 (A+I) D^-1/2 h W.  Row scaling commutes
    with right matmuls, relu commutes with positive row scaling, and
    agg(h W) = agg(h) W, so:
      L1 (fused W_pre@W1): u' = (scatter(xs) + xs_dst) * dinv/S1;
          g1 = relu(u' @ (W_pre W1) + b1 + rank1(b_pre))
      L2: v' = (scatter(q(g1*dinv)) + ...) * dinv/S2;
          y  = relu(v' @ W2 + b2) @ W_post + b_post
  - Two SPMD launches; host rescales/requantizes g1 between them.
"""

import dataclasses
import ml_dtypes
import numpy as np

import concourse.bacc as bacc
import concourse.bass as bass
import concourse.tile as tile
import concourse.mybir as mybir
from concourse.bass_utils import run_bass_kernel_spmd
from concourse.masks import make_identity

P = 128
NSWQ = 4  # SWDGE queues
GCH = 8  # gather tiles per dma_gather call
f32 = mybir.dt.float32
bf16 = mybir.dt.bfloat16
fp8 = mybir.dt.float8e3  # e3m4
FP8_TAB = True  # gather tables in e3m4 (half bytes) vs bf16
gdt = fp8 if FP8_TAB else bf16
np_gdt = ml_dtypes.float8_e3m4 if FP8_TAB else ml_dtypes.bfloat16


def _qscale(absmax):
    """Power-of-2 scale putting absmax at ~12 (e3m4 max finite = 15.5)."""
    return float(2.0 ** np.floor(np.log2(12.0 / max(absmax, 1e-30))))
i16 = mybir.dt.int16
i32 = mybir.dt.int32


@dataclasses.dataclass
class Cfg:
    n_nodes: int = 50000
    d: int = 256
    nc: int = 8
    split: int = 32768
    dense_n: int = 512

    @property
    def nloc(self):
        return self.n_nodes // self.nc

    @property
    def nblk(self):
        return (self.nloc + P - 1) // P

    @property
    def npad(self):
        return self.nblk * P

    @property
    def nhi(self):
        # table B spans the last min(32768, n) rows (full int16 window)
        return min(self.split, self.n_nodes)


# ---------------------------------------------------------------- host prep


def _wrap16(vals):
    """[n] -> [128, n//16] int16: value i at [i%16, i//16], replicated x8."""
    w = vals.reshape(-1, 16).T
    return np.tile(w, (8, 1)).astype(np.int16)


def _prep_edges(cfg, src, dst):
    """Partition edges by dst owner; split per dst-block into two gather
    groups (table A = rows [0, split); table B = rows [baseB, n)).  The cut
    is chosen per block at an exact multiple of 128 edges in src-order when
    the overlap window [baseB, split) allows, so group A has no padding.

    Returns (TA, TB, per-core dict of idx/slot planes) with identical
    compile-time schedule (TA, TB) across cores.
    """
    nl, nb = cfg.nloc, cfg.nblk
    baseB = cfg.n_nodes - cfg.nhi
    owner = dst // nl
    loc = dst - owner * nl
    blk = loc // P
    slot = loc - blk * P

    key = owner * nb + blk
    nkeys = cfg.nc * nb
    n_cb = np.bincount(key, minlength=nkeys).reshape(cfg.nc, nb)
    cntlo = np.bincount(key[src < baseB], minlength=nkeys).reshape(cfg.nc, nb)
    cntA = np.bincount(key[src < cfg.split], minlength=nkeys).reshape(cfg.nc, nb)

    lo = -(-cntlo // P).max(axis=0)          # [nb] min feasible TA
    hi = (cntA // P).min(axis=0)             # [nb] max feasible TA
    TA = np.maximum(hi, lo)                  # prefer max A (A has no pad)
    feasible = lo <= hi
    # fallback for infeasible blocks: threshold split at `split`
    TA = np.where(feasible, TA, -(-cntA // P).max(axis=0))
    # B count per (c,b): n - A_count; A_count = TA*P if feasible else cntA
    A_cnt = np.where(feasible[None, :], np.minimum(TA[None, :] * P, n_cb),
                     cntA)
    B_cnt = n_cb - A_cnt
    TB = (-(-B_cnt // P)).max(axis=0)

    # global A/B layout: all blocks' A segments first, then all B segments,
    # so gather calls can span block boundaries within one table
    baseA = np.concatenate([[0], np.cumsum(TA * P)])[:-1]
    baseBs = int(TA.sum()) * P + np.concatenate([[0], np.cumsum(TB * P)])[:-1]
    gbase = np.stack([baseA, baseBs], axis=1).astype(np.int64)
    rows_total = int((TA.sum() + TB.sum()) * P)

    # rank edges within (core, block) by src (stable) to apply the cut
    order = np.lexsort((src, key))
    skey = key[order]
    group_start = np.concatenate(
        [[0], np.cumsum(np.bincount(skey, minlength=nkeys))])
    rank = np.arange(len(src)) - group_start[skey]

    ocore = owner[order]
    oblk = blk[order]
    acut = A_cnt[ocore, oblk]
    in_a = rank < acut
    grp = (~in_a).astype(np.int64)
    rowpos = np.where(in_a, rank, rank - acut)
    rows = gbase[oblk, grp] + rowpos
    idxv = (src[order] - grp * baseB).astype(np.int16)
    slotv = slot[order].astype(np.float32)
    assert (src[order][in_a] < cfg.split).all()
    assert (src[order][~in_a] >= baseB).all()

    idx_flat = np.zeros((cfg.nc, rows_total), np.int16)
    slot_flat = np.full((cfg.nc, rows_total), 300.0, np.float32)
    idx_flat[ocore, rows] = idxv
    slot_flat[ocore, rows] = slotv

    per_core = []
    for c in range(cfg.nc):
        idxp = _wrap16(idx_flat[c])
        slotp = slot_flat[c].reshape(-1, P).T.copy()
        per_core.append({"idxp": idxp, "slotp": slotp})
    return TA, TB, per_core


def _wrap_cols(vec, nblk, npad):
    """[npad] -> [128, nblk] with [p, b] = vec[b*128+p]."""
    v = np.zeros(npad, np.float32)
    v[: len(vec)] = vec
    return v.reshape(nblk, P).T.copy()


def _wrap_tab(tab, nblk, d):
    """[npad, d] -> [128, nblk*d] with [p, b*d:(b+1)*d] = tab[b*128+p]."""
    return tab.reshape(nblk, P, d).transpose(1, 0, 2).reshape(P, nblk * d)


def _wrap_w(W):
    """[d, d] -> [128, kd*kd*128] tile-packed for a single weight DMA."""
    d = W.shape[0]
    kd = d // P
    return (W.reshape(kd, P, kd, P).transpose(1, 0, 2, 3)
            .reshape(P, kd * kd * P).astype(ml_dtypes.bfloat16))


# ------------------------------------------------------------- device build


def build_launch(cfg, mode, TA, TB, has_bpre=False):
    """mode 1: out = relu(u' @ WA + b1 [+ rank1])   (writes g1, feat-major)
    mode 2: out = relu(v' @ W2 + b2) @ W_post + b_post  (writes y, feat-major)
    """
    nb, npad, d = cfg.nblk, cfg.npad, cfg.d
    ntiles = int((TA + TB).sum())
    nidxcol = ntiles * 8
    tmaxP = max(int((TA + TB).max()), 1)

    # global A/B tile layout (matches host): A tiles of all blocks first,
    # then all B tiles.  Gather calls span block boundaries, so nearly all
    # calls are full GCH tiles.
    totA = int(TA.sum())
    ta_base = np.concatenate([[0], np.cumsum(TA)])[:-1]
    tb_base = totA + np.concatenate([[0], np.cumsum(TB)])[:-1]

    # gather-call plan per group: (grp, t0, cn, icol); icol = t0 * 8
    calls = []
    for grp, gt0, gcnt in ((0, 0, totA), (1, totA, int(TB.sum()))):
        for c0 in range(0, gcnt, GCH):
            cn = min(GCH, gcnt - c0)
            calls.append((grp, gt0 + c0, cn, (gt0 + c0) * 8))
    assert sum(c[2] for c in calls) == ntiles

    # split the idx plane into chunks at call boundaries so early gathers
    # only wait on the first chunks; A and B ranges are chunked separately
    # and their DMAs interleaved (block 0 needs the head of both ranges)
    acol = totA * 8
    astarts = [c[3] for c in calls if c[0] == 0]
    bstarts = [c[3] for c in calls if c[0] == 1]

    def _cuts(starts, lo, hi, fracs):
        cs = [lo]
        for fr in fracs:
            tgt = lo + (hi - lo) * fr
            best = min(starts, key=lambda x: abs(x - tgt))
            if cs[-1] < best < hi:
                cs.append(best)
        cs.append(hi)
        return list(zip(cs[:-1], cs[1:]))

    a_ch = _cuts(astarts, 0, acol, [0.02, 0.10, 0.30, 0.60])
    b_ch = (_cuts(bstarts, acol, nidxcol, [0.04, 0.50])
            if acol < nidxcol else [])
    # phase 0 loads only the first A and B chunks (block 0 needs the head
    # of both ranges); the rest stream just-in-time from the gather-call
    # cursor, keeping the 8 DMA-completion sem lanes free early on.
    chunks0 = [a_ch[0]] + b_ch[:1]

    nc = bacc.Bacc("TRN2", target_bir_lowering=False, debug=False,
                   num_devices=cfg.nc, num_swdge_queues=NSWQ)

    tablo = nc.dram_tensor("tablo", [cfg.split, d], gdt, kind="ExternalInput")
    loctab = nc.dram_tensor("loctab", [P, nb * d], gdt, kind="ExternalInput")
    tabhi = nc.dram_tensor("tabhi", [cfg.nhi, d], gdt, kind="ExternalInput")
    idxp_d = nc.dram_tensor("idxp", [P, nidxcol], i16, kind="ExternalInput")
    slotp_d = nc.dram_tensor("slotp", [P, ntiles], f32, kind="ExternalInput")
    dinvw_d = nc.dram_tensor("dinvw", [P, nb], f32, kind="ExternalInput")
    nw = 1 if mode == 1 else 2
    kd0 = d // P
    w_d = [nc.dram_tensor(f"w{i}", [P, kd0 * kd0 * P], bf16,
                          kind="ExternalInput")
           for i in range(nw)]
    bias_d = [nc.dram_tensor(f"bias{i}", [P, d // P], f32, kind="ExternalInput")
              for i in range(nw)]
    if has_bpre:
        c1rep_d = nc.dram_tensor("c1rep", [P, npad], f32, kind="ExternalInput")
        v1w_d = nc.dram_tensor("v1w", [P, d // P], f32, kind="ExternalInput")
    kd = d // P  # feature k-tiles (2)
    out_d = nc.dram_tensor("out", [kd, P, npad], bf16, kind="ExternalOutput")

    with tile.TileContext(nc) as tc:
        with (
            tc.tile_pool(name="const", bufs=1) as cpool,
            tc.tile_pool(name="gA", bufs=8) as gApool,
            tc.tile_pool(name="gB", bufs=6) as gBpool,
            tc.tile_pool(name="pmat", bufs=3) as ppool,
            tc.tile_pool(name="work", bufs=3) as wpool,
            tc.tile_pool(name="zslab", bufs=2) as zpool,
            tc.tile_pool(name="apsum", bufs=3, space="PSUM") as apsum,
            tc.tile_pool(name="trpsum", bufs=2, space="PSUM") as trpsum,
            tc.tile_pool(name="dpsum", bufs=3, space="PSUM") as dpsum,
        ):
            # ---- warmup: tiny gathers on all queues absorb SWDGE ucode
            # cold-start while the idx planes stream in
            izero = cpool.tile([P, 8], i16)
            nc.gpsimd.memset(izero[:], 0)
            warm = cpool.tile([P, NSWQ, d], gdt)
            for q in range(NSWQ):
                nc.gpsimd.dma_gather(
                    out_ap=warm[:, q:q + 1, :], in_ap=tablo[:],
                    idxs_ap=izero[:],
                    num_idxs=P, num_idxs_reg=P, elem_size=d, queue_num=q)

            # ---- phase-0 constants: only what block 0 needs, so the 8
            # DMA-completion sem lanes recycle to the gather stream fast
            idxp_t = cpool.tile([P, nidxcol], i16)
            slotp_t = cpool.tile([P, ntiles], f32)
            dinvw_t = cpool.tile([P, nb], f32)
            loctab_t = cpool.tile([P, nb, d], gdt)
            for lo, hi in chunks0:
                nc.sync.dma_start(idxp_t[:, lo:hi], idxp_d[:, lo:hi])
            nc.sync.dma_start(slotp_t[:], slotp_d[:])
            nc.sync.dma_start(dinvw_t[:], dinvw_d[:])
            lsplit = min(3, nb)
            nc.scalar.dma_start(loctab_t[:, 0:lsplit, :],
                                loctab[:, 0:lsplit * d])

            def load_phase1():
                if lsplit < nb:
                    nc.scalar.dma_start(loctab_t[:, lsplit:nb, :],
                                        loctab[:, lsplit * d:nb * d])

            # remaining idx chunks stream just-in-time, driven by the
            # gather-call cursor with a lookahead (so each chunk's DMA is
            # issued in program order well before the first call reading it)
            pend = [sorted(a_ch[1:]), sorted(b_ch[1:])]
            LOOKC = GCH * 8 * 12

            def feed_chunks(grp, ic_end):
                while pend[grp] and pend[grp][0][0] < ic_end + LOOKC:
                    lo, hi = pend[grp].pop(0)
                    nc.sync.dma_start(idxp_t[:, lo:hi], idxp_d[:, lo:hi])
            iota_i = cpool.tile([P, P], i32)
            nc.gpsimd.iota(iota_i[:], pattern=[[1, P]], base=0,
                           channel_multiplier=0)
            iota_f = cpool.tile([P, P], f32)
            nc.vector.tensor_copy(iota_f[:], iota_i[:])
            ident = cpool.tile([P, P], f32)
            make_identity(nc, ident[:])
            ident_b = cpool.tile([P, P], bf16)
            nc.vector.tensor_copy(ident_b[:], ident[:])
            if has_bpre:
                c1rep_t = cpool.tile([P, npad], f32)
                nc.sync.dma_start(c1rep_t[:], c1rep_d[:])
                v1w_t = cpool.tile([P, kd], f32)
                nc.sync.dma_start(v1w_t[:], v1w_d[:])

            # weights/biases are loaded lazily (after block 0's gathers are
            # queued) on the scalar queue
            w_t = [None] * nw
            bias_t = [None] * nw

            def load_weights():
                for i in range(nw):
                    wall = cpool.tile([P, kd, kd, P], bf16, name=f"wall{i}",
                                      tag=f"wall{i}")
                    nc.scalar.dma_start(wall[:], w_d[i][:])
                    w_t[i] = wall
                    bt = cpool.tile([P, kd], f32, name=f"bt{i}", tag=f"bt{i}")
                    nc.scalar.dma_start(bt[:], bias_d[i][:])
                    bias_t[i] = bt

            # feature-major activations, one tile per dense node-slice
            nsl = (npad + cfg.dense_n - 1) // cfg.dense_n
            uT_s = [cpool.tile([P, kd, min(cfg.dense_n, npad - i * cfg.dense_n)],
                               bf16, name=f"uTs{i}", tag=f"uTs{i}")
                    for i in range(nsl)]

            def emit_dense(s):
                s0 = s * cfg.dense_n
                ns = min(cfg.dense_n, npad - s0)
                pz = [dpsum.tile([P, ns], f32, space="PSUM", tag="dps",
                                 name=f"pz{s}_{dt}") for dt in range(kd)]
                for dt in range(kd):
                    for m in range(kd):
                        nc.tensor.matmul(
                            pz[dt][:], lhsT=w_t[0][:, m, dt, :],
                            rhs=uT_s[s][:, m, 0:ns],
                            start=(m == 0), stop=(m == kd - 1))
                if has_bpre:
                    for dt in range(kd):
                        tmp = wpool.tile([P, cfg.dense_n], f32, tag="r1")
                        nc.vector.tensor_scalar_mul(
                            tmp[:, 0:ns], c1rep_t[:, s0:s0 + ns],
                            v1w_t[:, dt:dt + 1])
                        nc.vector.tensor_tensor(
                            out=pz[dt][:], in0=pz[dt][:], in1=tmp[:, 0:ns],
                            op=mybir.AluOpType.add)

                if mode == 1:
                    final = zpool.tile([P, kd, cfg.dense_n], bf16, tag="zr")
                    for dt in range(kd):
                        nc.scalar.activation(
                            final[:, dt, 0:ns], pz[dt][:],
                            mybir.ActivationFunctionType.Relu,
                            bias=bias_t[0][:, dt:dt + 1], scale=1.0)
                else:
                    rT = zpool.tile([P, kd, cfg.dense_n], bf16, tag="zr")
                    for dt in range(kd):
                        nc.scalar.activation(
                            rT[:, dt, 0:ns], pz[dt][:],
                            mybir.ActivationFunctionType.Relu,
                            bias=bias_t[0][:, dt:dt + 1], scale=1.0)
                    py = [dpsum.tile([P, ns], f32, space="PSUM", tag="dps",
                                     name=f"py{s}_{dt}") for dt in range(kd)]
                    for dt in range(kd):
                        for m in range(kd):
                            nc.tensor.matmul(
                                py[dt][:], lhsT=w_t[1][:, m, dt, :],
                                rhs=rT[:, m, 0:ns],
                                start=(m == 0), stop=(m == kd - 1))
                    final = zpool.tile([P, kd, cfg.dense_n], bf16, tag="yT")
                    for dt in range(kd):
                        nc.scalar.activation(
                            final[:, dt, 0:ns], py[dt][:],
                            mybir.ActivationFunctionType.Identity,
                            bias=bias_t[1][:, dt:dt + 1], scale=1.0)

                for dt in range(kd):
                    nc.scalar.dma_start(out_d[dt, :, s0:s0 + ns],
                                        final[:, dt, 0:ns])

            # ---- aggregation pass with dense slices interleaved (LAG
            # blocks after a slice's last aggregation block)
            LAG = 2
            qload = [0] * NSWQ  # greedy row-balance across SWDGE queues
            ci = [0, 0]  # per-group next-call cursor (A calls precede B)
            ncallA = len([c for c in calls if c[0] == 0])
            gtiles = [[], []]  # per-group list of emitted gt tile handles

            def ensure_call(grp, t):
                # emit group-grp gather calls (in order) until tile t is
                # covered; consumption is sequential so this emits <= 1
                # call per steady-state invocation
                base = 0 if grp == 0 else ncallA
                ncg = ncallA if grp == 0 else len(calls) - ncallA
                while ci[grp] < ncg:
                    _, t0, cn, ic = calls[base + ci[grp]]
                    if t < t0:
                        return
                    feed_chunks(grp, ic + cn * 8)
                    pool_g = gApool if grp == 0 else gBpool
                    tab_ap = tablo if grp == 0 else tabhi
                    gtag = "gA" if grp == 0 else "gB"
                    gt = pool_g.tile([P, GCH, d], gdt, tag=gtag,
                                     name=f"g_{gtag}_{ci[grp]}")
                    q = qload.index(min(qload))
                    nc.gpsimd.dma_gather(
                        out_ap=gt[:, 0:cn, :], in_ap=tab_ap[:],
                        idxs_ap=idxp_t[:, ic:ic + cn * 8],
                        num_idxs=cn * P, num_idxs_reg=cn * P, elem_size=d,
                        queue_num=q)
                    qload[q] += cn
                    gtiles[grp].append(gt)
                    ci[grp] += 1
                    if t < t0 + cn:
                        return

            next_s = 0
            for b in range(nb):
                ta, tb = int(TA[b]), int(TB[b])
                tbt = ta + tb
                tA0, tB0 = int(ta_base[b]), int(tb_base[b])
                psum_a = apsum.tile([P, d], f32, space="PSUM", tag="psum_a")
                if tbt:
                    p_all = ppool.tile([P, tmaxP, P], bf16, tag="pmat")
                    if ta:
                        nc.vector.tensor_tensor(
                            out=p_all[:, 0:ta, :],
                            in0=slotp_t[:, tA0:tA0 + ta, None].to_broadcast(
                                [P, ta, P]),
                            in1=iota_f[:, None, :].to_broadcast([P, ta, P]),
                            op=mybir.AluOpType.is_equal)
                    if tb:
                        nc.vector.tensor_tensor(
                            out=p_all[:, ta:tbt, :],
                            in0=slotp_t[:, tB0:tB0 + tb, None].to_broadcast(
                                [P, tb, P]),
                            in1=iota_f[:, None, :].to_broadcast([P, tb, P]),
                            op=mybir.AluOpType.is_equal)
                nc.tensor.matmul(psum_a[:], lhsT=ident_b[:],
                                 rhs=loctab_t[:, b, :],
                                 start=True, stop=(tbt == 0))
                for j in range(tbt):
                    grp = 0 if j < ta else 1
                    t = tA0 + j if grp == 0 else tB0 + (j - ta)
                    ensure_call(grp, t)
                    off = 0 if grp == 0 else totA
                    gt = gtiles[grp][(t - off) // GCH]
                    nc.tensor.matmul(
                        psum_a[:], lhsT=p_all[:, j, :],
                        rhs=gt[:, (t - off) % GCH, :],
                        start=False, stop=(j == tbt - 1))

                # epilogue: u' = psum * dinv/S (self-loops are gathered rows)
                u2 = wpool.tile([P, d], bf16, tag="u2")
                nc.scalar.mul(u2[:], psum_a[:], dinvw_t[:, b:b + 1])
                sl, off = divmod(b * P, cfg.dense_n)
                for m in range(kd):
                    ptr = trpsum.tile([P, P], bf16, space="PSUM", tag="ptr")
                    nc.tensor.transpose(out=ptr[:],
                                        in_=u2[:, m * P:(m + 1) * P],
                                        identity=ident_b[:])
                    nc.vector.tensor_copy(uT_s[sl][:, m, off:off + P], ptr[:])

                if b == min(1, nb - 1):
                    load_phase1()
                if b == min(4, nb - 1):
                    load_weights()
                while next_s < nsl and min(4 * next_s + 3, nb - 1) <= b - LAG:
                    emit_dense(next_s)
                    next_s += 1

            while next_s < nsl:
                emit_dense(next_s)
                next_s += 1

    nc.compile()
    return nc


# ------------------------------------------------------------------ driver


def _run(cfg, nc_prog, per_core_common, per_core_vars, trace=False):
    in_maps = []
    for c in range(cfg.nc):
        m = dict(per_core_common)
        m.update(per_core_vars[c])
        in_maps.append(m)
    res = run_bass_kernel_spmd(nc_prog, in_maps, core_ids=list(range(cfg.nc)),
                               trace=trace)
    return res


def gcn_forward(cfg, x, edge_index, W_pre, b_pre, W1, b1, W2, b2, W_post,
                b_post, trace=False, ret_times=None):
    x = np.asarray(x, np.float32)
    src = np.asarray(edge_index[0], np.int64)
    dst = np.asarray(edge_index[1], np.int64)
    W_pre, W1, W2, W_post = (np.asarray(w, np.float32)
                             for w in (W_pre, W1, W2, W_post))
    b_pre, b1, b2, b_post = (np.asarray(b, np.float32)
                             for b in (b_pre, b1, b2, b_post))

    n, d, nl, nb, npad = cfg.n_nodes, cfg.d, cfg.nloc, cfg.nblk, cfg.npad
    deg = (np.bincount(dst, minlength=n) + 1).astype(np.float64)
    dinv = (1.0 / np.sqrt(deg)).astype(np.float32)

    TA, TB, edge_planes = _prep_edges(cfg, src, dst)

    def local_pad(tab, c):
        out = np.zeros((npad, d), tab.dtype)
        out[:nl] = tab[c * nl:(c + 1) * nl]
        return _wrap_tab(out, nb, d)

    xs = x * dinv[:, None]
    WA = (W_pre.astype(np.float64) @ W1.astype(np.float64)).astype(np.float32)

    has_bpre = bool(np.any(b_pre != 0))

    def unpack_out(res, c):
        # [kd, P, npad] feature-major -> [nl, d]
        o = np.asarray(res.results[c]["out"])
        return o.reshape(d, npad).T[:nl]

    # ---------- launch 1
    prog1 = build_launch(cfg, 1, TA, TB, has_bpre=has_bpre)
    S1 = _qscale(float(np.abs(xs).max()))
    xq = (xs * np.float32(S1)).astype(np_gdt)
    dinv1 = (dinv / np.float32(S1)).astype(np.float32)
    common1 = {
        "tablo": xq[: cfg.split],
        "tabhi": xq[cfg.n_nodes - cfg.nhi:],
        "w0": _wrap_w(WA),
        "bias0": b1.reshape(d // P, P).T.copy(),
    }
    if has_bpre:
        v1 = (b_pre.astype(np.float64) @ W1.astype(np.float64)).astype(
            np.float32)
        common1["v1w"] = v1.reshape(d // P, P).T.copy()
        # c1[dst] = (s[dst] + dinv[dst]) * dinv[dst],  s = sum_e dinv[src]
        s = np.zeros(n, np.float64)
        np.add.at(s, dst, dinv[src].astype(np.float64))
        c1_full = ((s + dinv) * dinv).astype(np.float32)
    vars1 = []
    for c in range(cfg.nc):
        v = {
            "loctab": local_pad(xq, c),
            "idxp": edge_planes[c]["idxp"],
            "slotp": edge_planes[c]["slotp"],
            "dinvw": _wrap_cols(dinv1[c * nl:(c + 1) * nl], nb, npad),
        }
        if has_bpre:
            cl = np.zeros(npad, np.float32)
            cl[:nl] = c1_full[c * nl:(c + 1) * nl]
            v["c1rep"] = np.tile(cl, (P, 1))
        vars1.append(v)
    res1 = _run(cfg, prog1, common1, vars1, trace=trace)
    g1 = np.concatenate([unpack_out(res1, c) for c in range(cfg.nc)]
                        ).astype(np.float32)
    g1 *= dinv[:, None]
    if ret_times is not None:
        ret_times.append(res1.exec_time_ns)

    # ---------- launch 2
    prog2 = build_launch(cfg, 2, TA, TB, has_bpre=False)
    S2 = _qscale(float(np.abs(g1).max()))
    g1q = (g1 * np.float32(S2)).astype(np_gdt)
    dinv2 = (dinv / np.float32(S2)).astype(np.float32)
    common2 = {
        "tablo": g1q[: cfg.split],
        "tabhi": g1q[cfg.n_nodes - cfg.nhi:],
        "w0": _wrap_w(W2),
        "w1": _wrap_w(W_post),
        "bias0": b2.reshape(d // P, P).T.copy(),
        "bias1": b_post.reshape(d // P, P).T.copy(),
    }
    vars2 = []
    for c in range(cfg.nc):
        vars2.append({
            "loctab": local_pad(g1q, c),
            "idxp": edge_planes[c]["idxp"],
            "slotp": edge_planes[c]["slotp"],
            "dinvw": _wrap_cols(dinv2[c * nl:(c + 1) * nl], nb, npad),
        })
    res2 = _run(cfg, prog2, common2, vars2, trace=trace)
    y = np.concatenate([unpack_out(res2, c) for c in range(cfg.nc)]
                       ).astype(np.float32)
    if ret_times is not None:
        ret_times.append(res2.exec_time_ns)
    return y


def kernel(x, edge_index, W_pre, b_pre, W1, b1, W2, b2, W_post, b_post):
    cfg = Cfg()
    return gcn_forward(cfg, x, edge_index, W_pre, b_pre, W1, b1, W2, b2,
                       W_post, b_post)


# revision 31
# speedup vs baseline: 1.1571x; 1.0562x over previous
"""GCN NodeAttributeAggregator on 8 Trainium2 NeuronCores.

Strategy (node-sharded, dst-partitioned edges):
  - Host precomputes index metadata: degrees (with self-loops), dinv=rsqrt(deg),
    per-core edge lists partitioned by dst owner, grouped by (dst-block of 128,
    src<32768 vs src>=32768 for int16 gather indices), padded to 128-edge tiles.
  - Device per core: dma_gather rows of a pre-scaled node table (xs = x*dinv),
    scatter-by-matmul: one-hot P matrices (built on DVE via iota + is_equal)
    contract each 128-edge tile into a 128-dst PSUM accumulator.
  - Gather tables are fp8 e3m4 (256B rows — half the HBM/descriptor bytes of
    bf16); the power-of-2 quantization scale folds into the dinv epilogue.
  - Dense 256x256 matmuls run in bf16 feature-major (FWL full rate), with
    bf16 PE transposes at layout boundaries; outputs written feature-major
    in large contiguous DMAs (host untransposes).
  - Algebra: GCN layer out = D^-1/2 (A+I) D^-1/2 h W.  Row scaling commutes
    with right matmuls, relu commutes with positive row scaling, and
    agg(h W) = agg(h) W, so:
      L1 (fused W_pre@W1): u' = (scatter(xs) + xs_dst) * dinv/S1;
          g1 = relu(u' @ (W_pre W1) + b1 + rank1(b_pre))
      L2: v' = (scatter(q(g1*dinv)) + ...) * dinv/S2;
          y  = relu(v' @ W2 + b2) @ W_post + b_post
  - Two SPMD launches; host rescales/requantizes g1 between them.
"""

import dataclasses
import ml_dtypes
import numpy as np

import concourse.bacc as bacc
import concourse.bass as bass
import concourse.tile as tile
import concourse.mybir as mybir
from concourse.bass_utils import run_bass_kernel_spmd
from concourse.masks import make_identity

P = 128
NSWQ = 4  # SWDGE queues
GCH = 8  # gather tiles per dma_gather call
f32 = mybir.dt.float32
bf16 = mybir.dt.bfloat16
fp8 = mybir.dt.float8e3  # e3m4
FP8_TAB = True  # gather tables in e3m4 (half bytes) vs bf16
gdt = fp8 if FP8_TAB else bf16
np_gdt = ml_dtypes.float8_e3m4 if FP8_TAB else ml_dtypes.bfloat16


def _qscale(absmax):
    """Power-of-2 scale putting absmax at ~12 (e3m4 max finite = 15.5)."""
    return float(2.0 ** np.floor(np.log2(12.0 / max(absmax, 1e-30))))
i16 = mybir.dt.int16
i32 = mybir.dt.int32


@dataclasses.dataclass
class Cfg:
    n_nodes: int = 50000
    d: int = 256
    nc: int = 8
    split: int = 32768
    dense_n: int = 512

    @property
    def nloc(self):
        return self.n_nodes // self.nc

    @property
    def nblk(self):
        return (self.nloc + P - 1) // P

    @property
    def npad(self):
        return self.nblk * P

    @property
    def nhi(self):
        # table B spans the last min(32768, n) rows (full int16 window)
        return min(self.split, self.n_nodes)


# ---------------------------------------------------------------- host prep


def _wrap16(vals):
    """[n] -> [128, n//16] int16: value i at [i%16, i//16], replicated x8."""
    w = vals.reshape(-1, 16).T
    return np.tile(w, (8, 1)).astype(np.int16)


def _prep_edges(cfg, src, dst):
    """Partition edges by dst owner; split per dst-block into two gather
    groups (table A = rows [0, split); table B = rows [baseB, n)).  The cut
    is chosen per block at an exact multiple of 128 edges in src-order when
    the overlap window [baseB, split) allows, so group A has no padding.

    Returns (TA, TB, per-core dict of idx/slot planes) with identical
    compile-time schedule (TA, TB) across cores.
    """
    nl, nb = cfg.nloc, cfg.nblk
    baseB = cfg.n_nodes - cfg.nhi
    owner = dst // nl
    loc = dst - owner * nl
    blk = loc // P
    slot = loc - blk * P

    key = owner * nb + blk
    nkeys = cfg.nc * nb
    n_cb = np.bincount(key, minlength=nkeys).reshape(cfg.nc, nb)
    cntlo = np.bincount(key[src < baseB], minlength=nkeys).reshape(cfg.nc, nb)
    cntA = np.bincount(key[src < cfg.split], minlength=nkeys).reshape(cfg.nc, nb)

    lo = -(-cntlo // P).max(axis=0)          # [nb] min feasible TA
    hi = (cntA // P).min(axis=0)             # [nb] max feasible TA
    TA = np.maximum(hi, lo)                  # prefer max A (A has no pad)
    feasible = lo <= hi
    # fallback for infeasible blocks: threshold split at `split`
    TA = np.where(feasible, TA, -(-cntA // P).max(axis=0))
    # B count per (c,b): n - A_count; A_count = TA*P if feasible else cntA
    A_cnt = np.where(feasible[None, :], np.minimum(TA[None, :] * P, n_cb),
                     cntA)
    B_cnt = n_cb - A_cnt
    TB = (-(-B_cnt // P)).max(axis=0)

    # global A/B layout: all blocks' A segments first, then all B segments,
    # so gather calls can span block boundaries within one table
    baseA = np.concatenate([[0], np.cumsum(TA * P)])[:-1]
    baseBs = int(TA.sum()) * P + np.concatenate([[0], np.cumsum(TB * P)])[:-1]
    gbase = np.stack([baseA, baseBs], axis=1).astype(np.int64)
    rows_total = int((TA.sum() + TB.sum()) * P)

    # rank edges within (core, block) by src (stable) to apply the cut
    order = np.lexsort((src, key))
    skey = key[order]
    group_start = np.concatenate(
        [[0], np.cumsum(np.bincount(skey, minlength=nkeys))])
    rank = np.arange(len(src)) - group_start[skey]

    ocore = owner[order]
    oblk = blk[order]
    acut = A_cnt[ocore, oblk]
    in_a = rank < acut
    grp = (~in_a).astype(np.int64)
    rowpos = np.where(in_a, rank, rank - acut)
    rows = gbase[oblk, grp] + rowpos
    idxv = (src[order] - grp * baseB).astype(np.int16)
    slotv = slot[order].astype(np.float32)
    assert (src[order][in_a] < cfg.split).all()
    assert (src[order][~in_a] >= baseB).all()

    idx_flat = np.zeros((cfg.nc, rows_total), np.int16)
    slot_flat = np.full((cfg.nc, rows_total), 300.0, np.float32)
    idx_flat[ocore, rows] = idxv
    slot_flat[ocore, rows] = slotv

    per_core = []
    for c in range(cfg.nc):
        idxp = _wrap16(idx_flat[c])
        slotp = slot_flat[c].reshape(-1, P).T.copy()
        per_core.append({"idxp": idxp, "slotp": slotp})
    return TA, TB, per_core


def _wrap_cols(vec, nblk, npad):
    """[npad] -> [128, nblk] with [p, b] = vec[b*128+p]."""
    v = np.zeros(npad, np.float32)
    v[: len(vec)] = vec
    return v.reshape(nblk, P).T.copy()


def _wrap_tab(tab, nblk, d):
    """[npad, d] -> [128, nblk*d] with [p, b*d:(b+1)*d] = tab[b*128+p]."""
    return tab.reshape(nblk, P, d).transpose(1, 0, 2).reshape(P, nblk * d)


def _wrap_w(W):
    """[d, d] -> [128, kd*kd*128] tile-packed for a single weight DMA."""
    d = W.shape[0]
    kd = d // P
    return (W.reshape(kd, P, kd, P).transpose(1, 0, 2, 3)
            .reshape(P, kd * kd * P).astype(ml_dtypes.bfloat16))


# ------------------------------------------------------------- device build


def build_launch(cfg, mode, TA, TB, has_bpre=False):
    """mode 1: out = relu(u' @ WA + b1 [+ rank1])   (writes g1, feat-major)
    mode 2: out = relu(v' @ W2 + b2) @ W_post + b_post  (writes y, feat-major)
    """
    nb, npad, d = cfg.nblk, cfg.npad, cfg.d
    ntiles = int((TA + TB).sum())
    nidxcol = ntiles * 8
    tmaxP = max(int((TA + TB).max()), 1)

    # global A/B tile layout (matches host): A tiles of all blocks first,
    # then all B tiles.  Gather calls span block boundaries, so nearly all
    # calls are full GCH tiles.
    totA = int(TA.sum())
    ta_base = np.concatenate([[0], np.cumsum(TA)])[:-1]
    tb_base = totA + np.concatenate([[0], np.cumsum(TB)])[:-1]

    # gather-call plan per group: (grp, t0, cn, icol); icol = t0 * 8.
    # The first few calls ramp 2/4/6 tiles so the 4 queue contexts
    # de-phase (issues then spread out instead of arriving in lockstep
    # batches of 4 that let the DMA engines drain dry between batches).
    calls = []
    for grp, gt0, gcnt in ((0, 0, totA), (1, totA, int(TB.sum()))):
        c0 = 0
        for cn0 in ((2, 4, 6) if grp == 0 else ()):
            if c0 + cn0 >= gcnt:
                break
            calls.append((grp, gt0 + c0, cn0, (gt0 + c0) * 8))
            c0 += cn0
        while c0 < gcnt:
            cn = min(GCH, gcnt - c0)
            calls.append((grp, gt0 + c0, cn, (gt0 + c0) * 8))
            c0 += cn
    assert sum(c[2] for c in calls) == ntiles

    # split the idx plane into chunks at call boundaries so early gathers
    # only wait on the first chunks; A and B ranges are chunked separately
    # and their DMAs interleaved (block 0 needs the head of both ranges)
    acol = totA * 8
    astarts = [c[3] for c in calls if c[0] == 0]
    bstarts = [c[3] for c in calls if c[0] == 1]

    def _cuts(starts, lo, hi, fracs):
        cs = [lo]
        for fr in fracs:
            tgt = lo + (hi - lo) * fr
            best = min(starts, key=lambda x: abs(x - tgt))
            if cs[-1] < best < hi:
                cs.append(best)
        cs.append(hi)
        return list(zip(cs[:-1], cs[1:]))

    a_ch = _cuts(astarts, 0, acol, [0.02, 0.10, 0.30, 0.60])
    b_ch = (_cuts(bstarts, acol, nidxcol, [0.04, 0.50])
            if acol < nidxcol else [])
    # phase 0 loads only the first A and B chunks (block 0 needs the head
    # of both ranges); the rest stream just-in-time from the gather-call
    # cursor, keeping the 8 DMA-completion sem lanes free early on.
    chunks0 = [a_ch[0]] + b_ch[:1]

    nc = bacc.Bacc("TRN2", target_bir_lowering=False, debug=False,
                   num_devices=cfg.nc, num_swdge_queues=NSWQ)

    tablo = nc.dram_tensor("tablo", [cfg.split, d], gdt, kind="ExternalInput")
    loctab = nc.dram_tensor("loctab", [P, nb * d], gdt, kind="ExternalInput")
    tabhi = nc.dram_tensor("tabhi", [cfg.nhi, d], gdt, kind="ExternalInput")
    idxp_d = nc.dram_tensor("idxp", [P, nidxcol], i16, kind="ExternalInput")
    slotp_d = nc.dram_tensor("slotp", [P, ntiles], f32, kind="ExternalInput")
    dinvw_d = nc.dram_tensor("dinvw", [P, nb], f32, kind="ExternalInput")
    nw = 1 if mode == 1 else 2
    kd0 = d // P
    w_d = [nc.dram_tensor(f"w{i}", [P, kd0 * kd0 * P], bf16,
                          kind="ExternalInput")
           for i in range(nw)]
    bias_d = [nc.dram_tensor(f"bias{i}", [P, d // P], f32, kind="ExternalInput")
              for i in range(nw)]
    if has_bpre:
        c1rep_d = nc.dram_tensor("c1rep", [P, npad], f32, kind="ExternalInput")
        v1w_d = nc.dram_tensor("v1w", [P, d // P], f32, kind="ExternalInput")
    kd = d // P  # feature k-tiles (2)
    out_d = nc.dram_tensor("out", [kd, P, npad], bf16, kind="ExternalOutput")

    with tile.TileContext(nc) as tc:
        with (
            tc.tile_pool(name="const", bufs=1) as cpool,
            tc.tile_pool(name="gA", bufs=8) as gApool,
            tc.tile_pool(name="gB", bufs=6) as gBpool,
            tc.tile_pool(name="pmat", bufs=3) as ppool,
            tc.tile_pool(name="work", bufs=3) as wpool,
            tc.tile_pool(name="zslab", bufs=2) as zpool,
            tc.tile_pool(name="apsum", bufs=3, space="PSUM") as apsum,
            tc.tile_pool(name="trpsum", bufs=2, space="PSUM") as trpsum,
            tc.tile_pool(name="dpsum", bufs=3, space="PSUM") as dpsum,
        ):
            # ---- warmup: tiny gathers on all queues absorb SWDGE ucode
            # cold-start while the idx planes stream in
            izero = cpool.tile([P, 8], i16)
            nc.gpsimd.memset(izero[:], 0)
            warm = cpool.tile([P, NSWQ, d], gdt)
            for q in range(NSWQ):
                nc.gpsimd.dma_gather(
                    out_ap=warm[:, q:q + 1, :], in_ap=tablo[:],
                    idxs_ap=izero[:],
                    num_idxs=P, num_idxs_reg=P, elem_size=d, queue_num=q)

            # ---- phase-0 constants: only what block 0 needs, so the 8
            # DMA-completion sem lanes recycle to the gather stream fast
            idxp_t = cpool.tile([P, nidxcol], i16)
            slotp_t = cpool.tile([P, ntiles], f32)
            dinvw_t = cpool.tile([P, nb], f32)
            loctab_t = cpool.tile([P, nb, d], gdt)
            for lo, hi in chunks0:
                nc.sync.dma_start(idxp_t[:, lo:hi], idxp_d[:, lo:hi])
            nc.sync.dma_start(slotp_t[:], slotp_d[:])
            nc.sync.dma_start(dinvw_t[:], dinvw_d[:])
            lsplit = min(3, nb)
            nc.scalar.dma_start(loctab_t[:, 0:lsplit, :],
                                loctab[:, 0:lsplit * d])

            def load_phase1():
                if lsplit < nb:
                    nc.scalar.dma_start(loctab_t[:, lsplit:nb, :],
                                        loctab[:, lsplit * d:nb * d])

            # remaining idx chunks stream just-in-time, driven by the
            # gather-call cursor with a lookahead (so each chunk's DMA is
            # issued in program order well before the first call reading it)
            pend = [sorted(a_ch[1:]), sorted(b_ch[1:])]
            LOOKC = GCH * 8 * 12

            def feed_chunks(grp, ic_end):
                while pend[grp] and pend[grp][0][0] < ic_end + LOOKC:
                    lo, hi = pend[grp].pop(0)
                    nc.sync.dma_start(idxp_t[:, lo:hi], idxp_d[:, lo:hi])
            iota_i = cpool.tile([P, P], i32)
            nc.gpsimd.iota(iota_i[:], pattern=[[1, P]], base=0,
                           channel_multiplier=0)
            iota_f = cpool.tile([P, P], f32)
            nc.vector.tensor_copy(iota_f[:], iota_i[:])
            ident = cpool.tile([P, P], f32)
            make_identity(nc, ident[:])
            ident_b = cpool.tile([P, P], bf16)
            nc.vector.tensor_copy(ident_b[:], ident[:])
            if has_bpre:
                c1rep_t = cpool.tile([P, npad], f32)
                nc.sync.dma_start(c1rep_t[:], c1rep_d[:])
                v1w_t = cpool.tile([P, kd], f32)
                nc.sync.dma_start(v1w_t[:], v1w_d[:])

            # weights/biases are loaded lazily (after block 0's gathers are
            # queued) on the scalar queue
            w_t = [None] * nw
            bias_t = [None] * nw

            def load_weights():
                for i in range(nw):
                    wall = cpool.tile([P, kd, kd, P], bf16, name=f"wall{i}",
                                      tag=f"wall{i}")
                    nc.scalar.dma_start(wall[:], w_d[i][:])
                    w_t[i] = wall
                    bt = cpool.tile([P, kd], f32, name=f"bt{i}", tag=f"bt{i}")
                    nc.scalar.dma_start(bt[:], bias_d[i][:])
                    bias_t[i] = bt

            # feature-major activations, one tile per dense node-slice
            nsl = (npad + cfg.dense_n - 1) // cfg.dense_n
            uT_s = [cpool.tile([P, kd, min(cfg.dense_n, npad - i * cfg.dense_n)],
                               bf16, name=f"uTs{i}", tag=f"uTs{i}")
                    for i in range(nsl)]

            def emit_dense(s):
                s0 = s * cfg.dense_n
                ns = min(cfg.dense_n, npad - s0)
                pz = [dpsum.tile([P, ns], f32, space="PSUM", tag="dps",
                                 name=f"pz{s}_{dt}") for dt in range(kd)]
                for dt in range(kd):
                    for m in range(kd):
                        nc.tensor.matmul(
                            pz[dt][:], lhsT=w_t[0][:, m, dt, :],
                            rhs=uT_s[s][:, m, 0:ns],
                            start=(m == 0), stop=(m == kd - 1))
                if has_bpre:
                    for dt in range(kd):
                        tmp = wpool.tile([P, cfg.dense_n], f32, tag="r1")
                        nc.vector.tensor_scalar_mul(
                            tmp[:, 0:ns], c1rep_t[:, s0:s0 + ns],
                            v1w_t[:, dt:dt + 1])
                        nc.vector.tensor_tensor(
                            out=pz[dt][:], in0=pz[dt][:], in1=tmp[:, 0:ns],
                            op=mybir.AluOpType.add)

                if mode == 1:
                    final = zpool.tile([P, kd, cfg.dense_n], bf16, tag="zr")
                    for dt in range(kd):
                        nc.scalar.activation(
                            final[:, dt, 0:ns], pz[dt][:],
                            mybir.ActivationFunctionType.Relu,
                            bias=bias_t[0][:, dt:dt + 1], scale=1.0)
                else:
                    rT = zpool.tile([P, kd, cfg.dense_n], bf16, tag="zr")
                    for dt in range(kd):
                        nc.scalar.activation(
                            rT[:, dt, 0:ns], pz[dt][:],
                            mybir.ActivationFunctionType.Relu,
                            bias=bias_t[0][:, dt:dt + 1], scale=1.0)
                    py = [dpsum.tile([P, ns], f32, space="PSUM", tag="dps",
                                     name=f"py{s}_{dt}") for dt in range(kd)]
                    for dt in range(kd):
                        for m in range(kd):
                            nc.tensor.matmul(
                                py[dt][:], lhsT=w_t[1][:, m, dt, :],
                                rhs=rT[:, m, 0:ns],
                                start=(m == 0), stop=(m == kd - 1))
                    final = zpool.tile([P, kd, cfg.dense_n], bf16, tag="yT")
                    for dt in range(kd):
                        nc.scalar.activation(
                            final[:, dt, 0:ns], py[dt][:],
                            mybir.ActivationFunctionType.Identity,
                            bias=bias_t[1][:, dt:dt + 1], scale=1.0)

                for dt in range(kd):
                    nc.scalar.dma_start(out_d[dt, :, s0:s0 + ns],
                                        final[:, dt, 0:ns])

            # ---- aggregation pass with dense slices interleaved (LAG
            # blocks after a slice's last aggregation block)
            LAG = 2
            qload = [0] * NSWQ  # greedy row-balance across SWDGE queues
            ci = [0, 0]  # per-group next-call cursor (A calls precede B)
            ncallA = len([c for c in calls if c[0] == 0])
            gtiles = [[], []]  # per-group list of emitted gt tile handles
            # tile -> (call seq within its group, slot within call)
            tile2call = {}
            _seqs = [0, 0]
            for _g, _t0, _cn, _ in calls:
                for _k in range(_cn):
                    tile2call[_t0 + _k] = (_seqs[_g], _k)
                _seqs[_g] += 1

            def ensure_call(grp, t):
                # emit group-grp gather calls (in order) until tile t is
                # covered; consumption is sequential so this emits <= 1
                # call per steady-state invocation
                base = 0 if grp == 0 else ncallA
                ncg = ncallA if grp == 0 else len(calls) - ncallA
                while ci[grp] < ncg:
                    _, t0, cn, ic = calls[base + ci[grp]]
                    if t < t0:
                        return
                    feed_chunks(grp, ic + cn * 8)
                    pool_g = gApool if grp == 0 else gBpool
                    tab_ap = tablo if grp == 0 else tabhi
                    gtag = "gA" if grp == 0 else "gB"
                    gt = pool_g.tile([P, GCH, d], gdt, tag=gtag,
                                     name=f"g_{gtag}_{ci[grp]}")
                    q = qload.index(min(qload))
                    nc.gpsimd.dma_gather(
                        out_ap=gt[:, 0:cn, :], in_ap=tab_ap[:],
                        idxs_ap=idxp_t[:, ic:ic + cn * 8],
                        num_idxs=cn * P, num_idxs_reg=cn * P, elem_size=d,
                        queue_num=q, single_packet=False)
                    qload[q] += cn
                    gtiles[grp].append(gt)
                    ci[grp] += 1
                    if t < t0 + cn:
                        return

            next_s = 0
            for b in range(nb):
                ta, tb = int(TA[b]), int(TB[b])
                tbt = ta + tb
                tA0, tB0 = int(ta_base[b]), int(tb_base[b])
                psum_a = apsum.tile([P, d], f32, space="PSUM", tag="psum_a")
                if tbt:
                    p_all = ppool.tile([P, tmaxP, P], bf16, tag="pmat")
                    if ta:
                        nc.vector.tensor_tensor(
                            out=p_all[:, 0:ta, :],
                            in0=slotp_t[:, tA0:tA0 + ta, None].to_broadcast(
                                [P, ta, P]),
                            in1=iota_f[:, None, :].to_broadcast([P, ta, P]),
                            op=mybir.AluOpType.is_equal)
                    if tb:
                        nc.vector.tensor_tensor(
                            out=p_all[:, ta:tbt, :],
                            in0=slotp_t[:, tB0:tB0 + tb, None].to_broadcast(
                                [P, tb, P]),
                            in1=iota_f[:, None, :].to_broadcast([P, tb, P]),
                            op=mybir.AluOpType.is_equal)
                nc.tensor.matmul(psum_a[:], lhsT=ident_b[:],
                                 rhs=loctab_t[:, b, :],
                                 start=True, stop=(tbt == 0))
                for j in range(tbt):
                    grp = 0 if j < ta else 1
                    t = tA0 + j if grp == 0 else tB0 + (j - ta)
                    ensure_call(grp, t)
                    cidx, cslot = tile2call[t]
                    gt = gtiles[grp][cidx]
                    nc.tensor.matmul(
                        psum_a[:], lhsT=p_all[:, j, :],
                        rhs=gt[:, cslot, :],
                        start=False, stop=(j == tbt - 1))

                # epilogue: u' = psum * dinv/S (self-loops are gathered rows)
                u2 = wpool.tile([P, d], bf16, tag="u2")
                nc.scalar.mul(u2[:], psum_a[:], dinvw_t[:, b:b + 1])
                sl, off = divmod(b * P, cfg.dense_n)
                for m in range(kd):
                    ptr = trpsum.tile([P, P], bf16, space="PSUM", tag="ptr")
                    nc.tensor.transpose(out=ptr[:],
                                        in_=u2[:, m * P:(m + 1) * P],
                                        identity=ident_b[:])
                    nc.vector.tensor_copy(uT_s[sl][:, m, off:off + P], ptr[:])

                if b == min(1, nb - 1):
                    load_phase1()
                if b == min(4, nb - 1):
                    load_weights()
                while next_s < nsl and min(4 * next_s + 3, nb - 1) <= b - LAG:
                    emit_dense(next_s)
                    next_s += 1

            while next_s < nsl:
                emit_dense(next_s)
                next_s += 1

    nc.compile()
    return nc


# ------------------------------------------------------------------ driver


def _run(cfg, nc_prog, per_core_common, per_core_vars, trace=False):
    in_maps = []
    for c in range(cfg.nc):
        m = dict(per_core_common)
        m.update(per_core_vars[c])
        in_maps.append(m)
    res = run_bass_kernel_spmd(nc_prog, in_maps, core_ids=list(range(cfg.nc)),
                               trace=trace)
    return res


def gcn_forward(cfg, x, edge_index, W_pre, b_pre, W1, b1, W2, b2, W_post,
                b_post, trace=False, ret_times=None):
    x = np.asarray(x, np.float32)
    src = np.asarray(edge_index[0], np.int64)
    dst = np.asarray(edge_index[1], np.int64)
    W_pre, W1, W2, W_post = (np.asarray(w, np.float32)
                             for w in (W_pre, W1, W2, W_post))
    b_pre, b1, b2, b_post = (np.asarray(b, np.float32)
                             for b in (b_pre, b1, b2, b_post))

    n, d, nl, nb, npad = cfg.n_nodes, cfg.d, cfg.nloc, cfg.nblk, cfg.npad
    deg = (np.bincount(dst, minlength=n) + 1).astype(np.float64)
    dinv = (1.0 / np.sqrt(deg)).astype(np.float32)

    TA, TB, edge_planes = _prep_edges(cfg, src, dst)

    def local_pad(tab, c):
        out = np.zeros((npad, d), tab.dtype)
        out[:nl] = tab[c * nl:(c + 1) * nl]
        return _wrap_tab(out, nb, d)

    xs = x * dinv[:, None]
    WA = (W_pre.astype(np.float64) @ W1.astype(np.float64)).astype(np.float32)

    has_bpre = bool(np.any(b_pre != 0))

    def unpack_out(res, c):
        # [kd, P, npad] feature-major -> [nl, d]
        o = np.asarray(res.results[c]["out"])
        return o.reshape(d, npad).T[:nl]

    # ---------- launch 1
    prog1 = build_launch(cfg, 1, TA, TB, has_bpre=has_bpre)
    S1 = _qscale(float(np.abs(xs).max()))
    xq = (xs * np.float32(S1)).astype(np_gdt)
    dinv1 = (dinv / np.float32(S1)).astype(np.float32)
    common1 = {
        "tablo": xq[: cfg.split],
        "tabhi": xq[cfg.n_nodes - cfg.nhi:],
        "w0": _wrap_w(WA),
        "bias0": b1.reshape(d // P, P).T.copy(),
    }
    if has_bpre:
        v1 = (b_pre.astype(np.float64) @ W1.astype(np.float64)).astype(
            np.float32)
        common1["v1w"] = v1.reshape(d // P, P).T.copy()
        # c1[dst] = (s[dst] + dinv[dst]) * dinv[dst],  s = sum_e dinv[src]
        s = np.zeros(n, np.float64)
        np.add.at(s, dst, dinv[src].astype(np.float64))
        c1_full = ((s + dinv) * dinv).astype(np.float32)
    vars1 = []
    for c in range(cfg.nc):
        v = {
            "loctab": local_pad(xq, c),
            "idxp": edge_planes[c]["idxp"],
            "slotp": edge_planes[c]["slotp"],
            "dinvw": _wrap_cols(dinv1[c * nl:(c + 1) * nl], nb, npad),
        }
        if has_bpre:
            cl = np.zeros(npad, np.float32)
            cl[:nl] = c1_full[c * nl:(c + 1) * nl]
            v["c1rep"] = np.tile(cl, (P, 1))
        vars1.append(v)
    res1 = _run(cfg, prog1, common1, vars1, trace=trace)
    g1 = np.concatenate([unpack_out(res1, c) for c in range(cfg.nc)]
                        ).astype(np.float32)
    g1 *= dinv[:, None]
    if ret_times is not None:
        ret_times.append(res1.exec_time_ns)

    # ---------- launch 2
    prog2 = build_launch(cfg, 2, TA, TB, has_bpre=False)
    S2 = _qscale(float(np.abs(g1).max()))
    g1q = (g1 * np.float32(S2)).astype(np_gdt)
    dinv2 = (dinv / np.float32(S2)).astype(np.float32)
    common2 = {
        "tablo": g1q[: cfg.split],
        "tabhi": g1q[cfg.n_nodes - cfg.nhi:],
        "w0": _wrap_w(W2),
        "w1": _wrap_w(W_post),
        "bias0": b2.reshape(d // P, P).T.copy(),
        "bias1": b_post.reshape(d // P, P).T.copy(),
    }
    vars2 = []
    for c in range(cfg.nc):
        vars2.append({
            "loctab": local_pad(g1q, c),
            "idxp": edge_planes[c]["idxp"],
            "slotp": edge_planes[c]["slotp"],
            "dinvw": _wrap_cols(dinv2[c * nl:(c + 1) * nl], nb, npad),
        })
    res2 = _run(cfg, prog2, common2, vars2, trace=trace)
    y = np.concatenate([unpack_out(res2, c) for c in range(cfg.nc)]
                       ).astype(np.float32)
    if ret_times is not None:
        ret_times.append(res2.exec_time_ns)
    return y


def kernel(x, edge_index, W_pre, b_pre, W1, b1, W2, b2, W_post, b_post):
    cfg = Cfg()
    return gcn_forward(cfg, x, edge_index, W_pre, b_pre, W1, b1, W2, b2,
                       W_post, b_post)
